# revision 27
# baseline (speedup 1.0000x reference)
"""Trainium2 Bass kernel for nn_AttentionBlock (linear attention + BatchNorm).

Math (per batch, c=256 channels, n=1024 pixels, 8 heads x 64 dims):
  qkv = w_qkv @ x                      [1536, n]
  q   = softmax(q, axis=d) * d^-0.5    (per head, over the 64 head-dims)
  k   = softmax(k, axis=n)             (per head-dim, over pixels)
  ctx = k @ (v/n)^T                    [d, e] per head
  out = ctx^T @ q                      [e, n] per head
  y   = BatchNorm(w_out @ out + b_out) (batch stats over (b, n) per channel)

Sharding: data-parallel over batch across 8 cores (4 batches each); BN batch
stats are combined with a tiny AllReduce (2 floats per channel). b_out is
skipped: BatchNorm's mean subtraction cancels any per-channel constant exactly.

Device layouts (per batch):
  x      [c, n]           c on partitions (2 tiles, fp16)
  kv     [n, (k|v)]       per n-tile one 2-bank PSUM tile: k cols 0:512,
                          v cols 512:1024.  exp(k) -> expk fp16 (ACT);
                          v -> vx [128,8,65] fp16 with a ones column.
  ctx_h  [d, e+1]         contraction over n (8 chunks, PSUM accum, 4 banks);
                          col 64 = Zk (ones column) -> per-partition norm
  q      [(h d), n]       one 2-bank PSUM tile per q-tile; ONE exp -> expq
  Zq     [(h d), n]       block-mask matmul; recip folds SCALE and 1/n
  out_h  [e, n]           lhsT=cs, rhs=expq, quadrant-packed pairs of heads,
                          2-bank PSUM; normalized by recipb -> os fp16
  final  [c, n]           lhsT=w_out^T; fs fp16 resident; bn_stats per (b,m);
                          AllReduce of packed (mean, E[x^2]); normalize in
                          place; DMA out fp16.
"""

import os
import sys

import numpy as np

for _p in ("/opt/trn_rl_repo", "/root/.axon_site/_ro/trn_rl_repo"):
    if os.path.isdir(_p) and _p not in sys.path:
        sys.path.insert(0, _p)

import concourse.bacc as bacc
import concourse.tile as tile
from concourse import mybir
from concourse.bass_utils import run_bass_kernel_spmd

F32 = mybir.dt.float32
FP16 = mybir.dt.float16
AF = mybir.ActivationFunctionType
ALU = mybir.AluOpType

N_CORES = 8
# B is overridable for cheap simulator runs (BASS_ATTN_B=1 -> 8 batches total).
B = int(os.environ.get("BASS_ATTN_B", "4"))  # batches per core
C = 256          # channels
NPIX = 1024      # pixels (32*32)
H = 8            # heads
D = 64           # head dim
HID = H * D      # 512
NT = NPIX // 128  # 8 n-tiles
CT = C // 128     # 2 c-tiles
QT = HID // 128   # 4 q-tiles
SCALE = D ** -0.5
BN_EPS = 1e-5
# Zq-broadcast matmul uses this instead of 1.0 so reciprocal(Zqb) directly
# yields SCALE / (n * Zq), folding the softmax scale and the v/n factor.
MASKVAL = NPIX / SCALE
N_WARM_MM = 24   # junk matmuls at t=0 keep PE busy so HAM un-throttles early


DEBUG = os.environ.get("BASS_ATTN_DEBUG") == "1"


def _emit(tc, x, wqkv, wout, gbuf, y, dbg=None):
    nc = tc.nc
    from contextlib import ExitStack
    ctx_stack = ExitStack()
    with ctx_stack:
        const = ctx_stack.enter_context(tc.tile_pool(name="const", bufs=1))
        kvsb = ctx_stack.enter_context(tc.tile_pool(name="kvsb", bufs=3))
        vxp = ctx_stack.enter_context(tc.tile_pool(name="vxp", bufs=3))
        qpool = ctx_stack.enter_context(tc.tile_pool(name="qpool", bufs=3))
        rpool = ctx_stack.enter_context(tc.tile_pool(name="rpool", bufs=3))
        cpool = ctx_stack.enter_context(tc.tile_pool(name="cpool", bufs=5))
        opool = ctx_stack.enter_context(tc.tile_pool(name="opool", bufs=6))
        fpool = ctx_stack.enter_context(tc.tile_pool(name="fpool", bufs=2 * B))
        small = ctx_stack.enter_context(tc.tile_pool(name="small", bufs=8))
        stats_p = ctx_stack.enter_context(tc.tile_pool(name="statsp", bufs=1))
        # PSUM: pbig 2 x 2-bank tiles + pctx 4 x 1-bank tiles = 8 banks
        pbig = ctx_stack.enter_context(
            tc.tile_pool(name="pbig", bufs=2, space="PSUM"))
        pctx = ctx_stack.enter_context(
            tc.tile_pool(name="pctx", bufs=4, space="PSUM"))
        dpool = ctx_stack.enter_context(
            tc.tile_pool(name="dram", bufs=1, space="DRAM"))

        # ---- constants / warmup ----
        eps_sb = const.tile([128, 1], F32, name="eps")
        nc.vector.memset(eps_sb, BN_EPS)
        # load the Exp table set while input DMAs are in flight
        warm_ex = small.tile([1, 1], F32, name="warmex")
        nc.scalar.activation(out=warm_ex, in_=eps_sb[0:1, :], func=AF.Exp)

        bmask = const.tile([128, 128], FP16, name="bmask")
        nc.vector.memset(bmask, 0.0)
        nc.vector.memset(bmask[0:64, 0:64], MASKVAL)
        nc.vector.memset(bmask[64:128, 64:128], MASKVAL)

        # x batch 0 + kv weight columns first (they gate the first matmul)
        xin = [[None] * CT for _ in range(B)]
        wqkv_sb = [const.tile([128, 3 * HID], FP16, name=f"wqkv{kc}")
                   for kc in range(CT)]
        for kc in range(CT):
            xt = const.tile([128, NPIX], FP16, name=f"x0_{kc}")
            nc.sync.dma_start(out=xt, in_=x[0, 128 * kc:128 * (kc + 1), :])
            xin[0][kc] = xt
            nc.sync.dma_start(out=wqkv_sb[kc][:, HID:3 * HID],
                              in_=wqkv[128 * kc:128 * (kc + 1), HID:3 * HID])
        for kc in range(CT):
            nc.sync.dma_start(out=wqkv_sb[kc][:, 0:HID],
                              in_=wqkv[128 * kc:128 * (kc + 1), 0:HID])

        # junk matmuls: keep PE busy from ~t=0 so the HAM clock-gate opens
        # (K=8/8) before the first real matmul instead of ~10us in
        jw = pbig.tile([128, NPIX], F32, name="junk", tag="pb")
        for i in range(N_WARM_MM):
            nc.tensor.matmul(jw[:, 0:128], lhsT=bmask, rhs=bmask,
                             start=True, stop=True)

        # remaining weights / inputs on other queues
        wout_sb = const.tile([128, 4 * C], FP16, name="wout")
        nc.scalar.dma_start(out=wout_sb, in_=wout)
        for b in range(1, B):
            for kc in range(CT):
                xt = const.tile([128, NPIX], FP16, name=f"x{b}_{kc}")
                nc.scalar.dma_start(
                    out=xt, in_=x[b, 128 * kc:128 * (kc + 1), :])
                xin[b][kc] = xt
        gb_sb = const.tile([128, 4], F32, name="gb")
        nc.scalar.dma_start(out=gb_sb, in_=gbuf)

        # dummy collective issued up front: the first AllReduce pays a
        # ~24us ncfw rendezvous; running it early overlaps that with compute
        no_cc = os.environ.get("BASS_ATTN_NO_CC") == "1"
        if not no_cc:
            wrm_i = dpool.tile([128, 1], F32, name="wrm_i")
            wrm_o = dpool.tile([128, 1], F32, name="wrm_o")
            nc.sync.dma_start(out=wrm_i, in_=eps_sb)
            nc.gpsimd.collective_compute(
                "AllReduce", ALU.add,
                replica_groups=[list(range(N_CORES))],
                ins=[wrm_i.opt()], outs=[wrm_o.opt()])

        stats_sb = [stats_p.tile([128, 2 * B, 6], F32, name=f"stats{m}")
                    for m in range(CT)]
        final_sb = [[None] * CT for _ in range(B)]

        # block-diagonal stationary tiles for the out matmuls: off-diagonal
        # 64x64 blocks stay zero forever, so memset each rotating buffer once
        N_BD = 5
        bd_bufs = [cpool.tile([128, 128], FP16, name=f"bd{i}")
                   for i in range(N_BD)]
        for t_ in bd_bufs:
            nc.vector.memset(t_, 0.0)

        # deferred final-projection emitter: interleaved into the next
        # batch's kv phase so the PE never drains at batch boundaries
        def emit_final(b):
            for m in range(CT):
                fs = fpool.tile([128, NPIX], FP16, name="final")
                for nch in range(2):
                    fh = pctx.tile([128, 512], F32, name="fp", tag="pc")
                    for k4 in range(QT):
                        nc.tensor.matmul(
                            fh,
                            lhsT=wout_sb[:, k4 * C + 128 * m:
                                         k4 * C + 128 * (m + 1)],
                            rhs=os_sb[b][k4][:, 512 * nch:512 * (nch + 1)],
                            start=(k4 == 0), stop=(k4 == QT - 1))
                    fsh = fs[:, 512 * nch:512 * (nch + 1)]
                    nc.scalar.copy(fsh, fh)
                    nc.vector.bn_stats(
                        out=stats_sb[m][:, 2 * b + nch, :], in_=fsh)
                final_sb[b][m] = fs

        os_sb = [[None] * QT for _ in range(B)]

        for b in range(B):
            xc = xin[b]

            # ---- KV projection + context accumulation over n-chunks ----
            ctxu = [pctx.tile([128, 512], F32, name="ctxu", tag="pc")
                    for _ in range(4)]
            kvp = [None] * NT

            def emit_kv_mm(t):
                # kc outer: consecutive matmul pairs share the stationary
                # operand (the x chunk); k and v halves are separate
                # accumulation groups in separate banks
                hp = pbig.tile([128, NPIX], F32, name="kvp", tag="pb")
                for kc in range(CT):
                    for nch in range(2):
                        nc.tensor.matmul(
                            hp[:, 512 * nch:512 * (nch + 1)],
                            lhsT=xc[kc][:, 128 * t:128 * (t + 1)],
                            rhs=wqkv_sb[kc][:, HID + 512 * nch:
                                            HID + 512 * (nch + 1)],
                            start=(kc == 0), stop=(kc == CT - 1))
                kvp[t] = hp

            def emit_kv_drain(t):
                ek = kvsb.tile([128, HID], FP16, name="expk")
                nc.scalar.activation(out=ek, in_=kvp[t][:, 0:512],
                                     func=AF.Exp)
                vx = vxp.tile([128, H, D + 1], FP16, name="vx")
                nc.vector.memset(vx[:, :, D:D + 1], 1.0)
                nc.vector.tensor_copy(
                    vx[:, :, 0:D],
                    kvp[t][:, 512:1024].rearrange("p (h e) -> p h e", h=H))
                return ek, vx

            def emit_ctx_mm(t, ek, vx):
                # one matmul per head-pair: out [128, 130] holds
                # (d=h0 rows x [e_h0 | Zk_h0] cols 0:65) and
                # (d=h1 rows x [e_h1 | Zk_h1] cols 65:130); the cross-head
                # blocks are computed but never read.
                for pr in range(4):
                    nc.tensor.matmul(
                        ctxu[pr][:, 0:2 * (D + 1)],
                        lhsT=ek[:, 128 * pr:128 * (pr + 1)],
                        rhs=vx[:, 2 * pr:2 * (pr + 1), :],
                        start=(t == 0), stop=(t == NT - 1))

            # software-pipelined: kv matmuls for t+1 are emitted before the
            # ctx matmuls for t, so the PE queue never stalls on the drains.
            # The previous batch's final projection is interleaved here too.
            drains = [None] * NT
            emit_kv_mm(0)
            if b > 0:
                emit_final(b - 1)

            for t in range(NT):
                if t + 1 < NT:
                    emit_kv_mm(t + 1)
                drains[t] = emit_kv_drain(t)
                emit_ctx_mm(t, *drains[t])

            # ---- context normalization (per-partition Zk) into the
            # block-diagonal out-matmul stationary ----
            cs = []
            for pr in range(4):
                if DEBUG and b == 0:
                    ct = small.tile([128, 2 * (D + 1)], F32, name="dbgct")
                    nc.vector.tensor_copy(ct, ctxu[pr][:, 0:2 * (D + 1)])
                    nc.sync.dma_start(out=dbg["ctx"][pr], in_=ct)
                rz = small.tile([128, 1], F32, name="rz")
                # the cross-head block's ones column at col 64 carries Zk for
                # BOTH heads (rows 0:64 = h0, rows 64:128 = h1)
                nc.vector.reciprocal_approx_fast(
                    out=rz, in_=ctxu[pr][:, D:D + 1])
                bd = bd_bufs[(b * 4 + pr) % N_BD]
                nc.vector.tensor_scalar_mul(
                    bd[0:64, 0:64], in0=ctxu[pr][0:64, 0:D],
                    scalar1=rz[0:64, :])
                nc.vector.tensor_scalar_mul(
                    bd[64:128, 64:128], in0=ctxu[pr][64:128, D + 1:2 * D + 1],
                    scalar1=rz[64:128, :])
                if DEBUG and b == 0:
                    nc.sync.dma_start(out=dbg["rz"][pr], in_=rz)
                cs.append(bd)

            # ---- Q projection, exp, Zq block-broadcast, reciprocal ----
            expq, recipb = [None] * QT, [None] * QT

            def emit_q_mm(t):
                qp = pbig.tile([128, NPIX], F32, name="qp", tag="pb")
                for kc in range(CT):
                    for nch in range(2):
                        nc.tensor.matmul(
                            qp[:, 512 * nch:512 * (nch + 1)],
                            lhsT=wqkv_sb[kc][:, 128 * t:128 * (t + 1)],
                            rhs=xc[kc][:, 512 * nch:512 * (nch + 1)],
                            start=(kc == 0), stop=(kc == CT - 1))
                return qp

            def emit_q_drain(t, qp):
                eq = qpool.tile([128, NPIX], FP16, name="expq")
                nc.scalar.activation(out=eq, in_=qp, func=AF.Exp)
                expq[t] = eq

            def emit_zq(t):
                rb = rpool.tile([128, NPIX], F32, name="recipb")
                for nch in range(2):
                    zp = pctx.tile([128, 512], F32, name="zq", tag="pc")
                    nc.tensor.matmul(
                        zp, lhsT=bmask,
                        rhs=expq[t][:, 512 * nch:512 * (nch + 1)],
                        start=True, stop=True)
                    nc.vector.reciprocal_approx_fast(
                        out=rb[:, 512 * nch:512 * (nch + 1)], in_=zp)
                recipb[t] = rb

            def emit_out(t):
                oh = pbig.tile([128, NPIX], F32, name="ou", tag="pb")
                for nch in range(2):
                    # block-diagonal stationary: one full-width matmul
                    # covers both heads of the pair
                    nc.tensor.matmul(
                        oh[:, 512 * nch:512 * (nch + 1)],
                        lhsT=cs[t],
                        rhs=expq[t][:, 512 * nch:512 * (nch + 1)],
                        start=True, stop=True)
                os_ = opool.tile([128, NPIX], FP16, name="outsb")
                nc.vector.tensor_mul(os_, oh, recipb[t])
                os_sb[b][t] = os_

            # pipeline: qmm(t+1) is emitted before zq(t) (which waits on the
            # ACT exp of tile t), and out(t) trails by one more stage
            qps = [None] * QT
            qps[0] = emit_q_mm(0)
            for t in range(QT):
                emit_q_drain(t, qps[t])
                if t + 1 < QT:
                    qps[t + 1] = emit_q_mm(t + 1)
                emit_zq(t)
                if t >= 1:
                    emit_out(t - 1)
            emit_out(QT - 1)

            if DEBUG and b == 0:
                nc.sync.dma_start(out=dbg["expk"], in_=drains[0][0])
                nc.sync.dma_start(out=dbg["vx"], in_=drains[0][1])
                for pr in range(4):
                    nc.sync.dma_start(out=dbg["bd"][pr], in_=cs[pr])
                    nc.sync.dma_start(out=dbg["os"][pr], in_=os_sb[b][pr])
                    nc.sync.dma_start(out=dbg["rb"][pr], in_=recipb[pr])
                    nc.sync.dma_start(out=dbg["eq"][pr], in_=expq[pr])


        emit_final(B - 1)

        # ---- batch-norm: aggregate, all-reduce, normalize, store ----
        ccin = dpool.tile([128, 2 * CT], F32, name="ccin")
        ccout = dpool.tile([128, 2 * CT], F32, name="ccout")
        # switch the ACT table to the sqrt set while PE still runs the last
        # final-proj matmuls, so the tail's Sqrt doesn't pay the ~2.7us load
        warm_sq = small.tile([1, 1], F32, name="warmsq")
        nc.scalar.activation(out=warm_sq, in_=eps_sb[0:1, :], func=AF.Sqrt)
        pk4 = small.tile([128, 2 * CT], F32, name="pk4")
        for m in range(CT):
            mv = small.tile([128, 2], F32, name="mv")
            nc.vector.bn_aggr(out=mv, in_=stats_sb[m])
            pk = pk4[:, 2 * m:2 * (m + 1)]
            nc.vector.tensor_mul(pk[:, 1:2], mv[:, 0:1], mv[:, 0:1])
            nc.vector.tensor_add(pk[:, 1:2], pk[:, 1:2], mv[:, 1:2])
            nc.vector.tensor_copy(pk[:, 0:1], mv[:, 0:1])
            nc.vector.tensor_scalar_mul(pk, in0=pk, scalar1=1.0 / N_CORES)
        nc.sync.dma_start(out=ccin, in_=pk4)
        if not no_cc:
            nc.gpsimd.collective_compute(
                "AllReduce", ALU.add,
                replica_groups=[list(range(N_CORES))],
                ins=[ccin.opt()], outs=[ccout.opt()])
        gst = small.tile([128, 2 * CT], F32, name="gst")
        nc.sync.dma_start(out=gst, in_=ccout if not no_cc else ccin)
        # both m-tiles' scale/shift computed in one [128, 2] op chain
        g2v = gst.rearrange("p (m k) -> p m k", k=2)
        gmean2 = g2v[:, :, 0]
        gex22 = g2v[:, :, 1]
        var2 = small.tile([128, 2], F32, name="var2")
        nc.vector.tensor_mul(var2, gmean2, gmean2)
        nc.vector.tensor_sub(var2, gex22, var2)
        std2 = small.tile([128, 2], F32, name="std2")
        nc.scalar.activation(out=std2, in_=var2, func=AF.Sqrt, bias=eps_sb)
        rsg2 = small.tile([128, 2], F32, name="rsg2")
        nc.vector.reciprocal_approx_fast(out=rsg2, in_=std2)
        nc.vector.tensor_mul(rsg2, rsg2, gb_sb[:, 0:2])
        sh2 = small.tile([128, 2], F32, name="sh2")
        nc.vector.tensor_mul(sh2, gmean2, rsg2)
        nc.vector.tensor_sub(sh2, gb_sb[:, 2:4], sh2)
        dmaq = [nc.sync, nc.gpsimd, nc.scalar, nc.gpsimd]
        for m in range(CT):
            rsg = rsg2[:, m:m + 1]
            sh = sh2[:, m:m + 1]
            for b in range(B):
                fs = final_sb[b][m]
                if b == 1:
                    nc.scalar.activation(
                        out=fs, in_=fs, func=AF.Identity,
                        bias=sh, scale=rsg)
                else:
                    nc.vector.tensor_scalar(
                        out=fs, in0=fs, scalar1=rsg, scalar2=sh,
                        op0=ALU.mult, op1=ALU.add)
                dmaq[(2 * m + b) % 4].dma_start(
                    out=y[b, 128 * m:128 * (m + 1), :], in_=fs)


_CACHE = {}


def _build():
    if "nc" in _CACHE:
        return _CACHE["nc"]
    nc = bacc.Bacc("TRN2", target_bir_lowering=False, debug=False,
                   enable_asserts=True, num_devices=N_CORES)
    x = nc.dram_tensor("x", [B, C, NPIX], FP16, kind="ExternalInput").ap()
    wqkv = nc.dram_tensor("wqkvT", [C, 3 * HID], FP16,
                          kind="ExternalInput").ap()
    wout = nc.dram_tensor("woutT", [128, 4 * C], FP16,
                          kind="ExternalInput").ap()
    gbuf = nc.dram_tensor("gbuf", [128, 4], F32, kind="ExternalInput").ap()
    y = nc.dram_tensor("y", [B, C, NPIX], FP16, kind="ExternalOutput").ap()
    dbg = None
    if DEBUG:
        dbg = {
            "expk": nc.dram_tensor("d_expk", [128, HID], FP16,
                                   kind="ExternalOutput").ap(),
            "vx": nc.dram_tensor("d_vx", [128, H, D + 1], FP16,
                                 kind="ExternalOutput").ap(),
            "bd": [nc.dram_tensor(f"d_bd{p}", [128, 128], FP16,
                                  kind="ExternalOutput").ap() for p in range(4)],
            "os": [nc.dram_tensor(f"d_os{p}", [128, NPIX], FP16,
                                  kind="ExternalOutput").ap() for p in range(4)],
            "rb": [nc.dram_tensor(f"d_rb{p}", [128, NPIX], F32,
                                  kind="ExternalOutput").ap() for p in range(4)],
            "eq": [nc.dram_tensor(f"d_eq{p}", [128, NPIX], FP16,
                                  kind="ExternalOutput").ap() for p in range(4)],
            "ctx": [nc.dram_tensor(f"d_ctx{p}", [128, 2 * (D + 1)], F32,
                                   kind="ExternalOutput").ap() for p in range(4)],
            "rz": [nc.dram_tensor(f"d_rz{p}", [128, 1], F32,
                                  kind="ExternalOutput").ap() for p in range(4)],
        }
    with tile.TileContext(nc) as tc:
        _emit(tc, x, wqkv, wout, gbuf, y, dbg)
    nc.compile()
    _CACHE["nc"] = nc
    return nc


def kernel(x, w_qkv, w_out, b_out, gamma, beta, _trace=False):
    x = np.asarray(x, dtype=np.float32)
    wqkvT = np.ascontiguousarray(np.asarray(w_qkv, np.float16).T)   # [256, 1536]
    woutT = np.asarray(w_out, np.float16).T                         # [512, 256]
    # [128, 4*256]: woutT[k4*128+p, c] -> [p, k4*256+c]
    woutp = np.ascontiguousarray(
        woutT.reshape(4, 128, C).transpose(1, 0, 2).reshape(128, 4 * C))
    # gbuf cols: gamma m=0, gamma m=1, beta m=0, beta m=1
    g2 = np.asarray(gamma, np.float32).reshape(CT, 128).T
    b2 = np.asarray(beta, np.float32).reshape(CT, 128).T
    gbuf = np.ascontiguousarray(np.concatenate([g2, b2], axis=1))
    # b_out is intentionally unused: BatchNorm's mean subtraction cancels any
    # per-channel constant added before it, exactly.

    btot, c, hh, ww = x.shape
    assert (btot, c, hh * ww) == (B * N_CORES, C, NPIX)
    xf = x.reshape(btot, C, NPIX)

    nc = _build()
    in_maps = []
    for core in range(N_CORES):
        in_maps.append({
            "x": np.ascontiguousarray(
                xf[B * core:B * (core + 1)]).astype(np.float16),
            "wqkvT": wqkvT,
            "woutT": woutp,
            "gbuf": gbuf,
        })
    res = run_bass_kernel_spmd(nc, in_maps, core_ids=list(range(N_CORES)),
                               trace=_trace)
    y = np.concatenate([np.asarray(res.results[core]["y"], np.float32)
                        for core in range(N_CORES)], axis=0)
    out = y.reshape(btot, C, hh, ww)
    if _trace:
        kernel.last_result = res
    return out


# revision 39
# speedup vs baseline: 1.1549x; 1.1549x over previous
"""Trainium2 Bass kernel for nn_AttentionBlock (linear attention + BatchNorm).

Math (per batch, c=256 channels, n=1024 pixels, 8 heads x 64 dims):
  qkv = w_qkv @ x                      [1536, n]
  q   = softmax(q, axis=d) * d^-0.5    (per head, over the 64 head-dims)
  k   = softmax(k, axis=n)             (per head-dim, over pixels)
  ctx = k @ (v/n)^T                    [d, e] per head
  out = ctx^T @ q                      [e, n] per head
  y   = BatchNorm(w_out @ out + b_out) (batch stats over (b, n) per channel)

Sharding: data-parallel over batch across 8 cores (4 batches each); BN batch
stats are combined with a tiny AllReduce (2 floats per channel). b_out is
skipped: BatchNorm's mean subtraction cancels any per-channel constant exactly.

Device layouts (per batch):
  x      [c, n]           c on partitions (2 tiles, fp16)
  kv     [n, (k|v)]       per n-tile one 2-bank PSUM tile: k cols 0:512,
                          v cols 512:1024.  exp(k) -> expk fp16 (ACT);
                          v -> vx [128,8,65] fp16 with a ones column.
  ctx_h  [d, e+1]         contraction over n (8 chunks, PSUM accum, 4 banks);
                          col 64 = Zk (ones column) -> per-partition norm
  q      [(h d), n]       one 2-bank PSUM tile per q-tile; ONE exp -> expq
  Zq     [(h d), n]       block-mask matmul; recip folds SCALE and 1/n
  out_h  [e, n]           lhsT=cs, rhs=expq, quadrant-packed pairs of heads,
                          2-bank PSUM; normalized by recipb -> os fp16
  final  [c, n]           lhsT=w_out^T; fs fp16 resident; bn_stats per (b,m);
                          AllReduce of packed (mean, E[x^2]); normalize in
                          place; DMA out fp16.
"""

import os
import sys

import numpy as np

for _p in ("/opt/trn_rl_repo", "/root/.axon_site/_ro/trn_rl_repo"):
    if os.path.isdir(_p) and _p not in sys.path:
        sys.path.insert(0, _p)

import concourse.bacc as bacc
import concourse.tile as tile
from concourse import mybir
from concourse.bass_utils import run_bass_kernel_spmd

F32 = mybir.dt.float32
FP16 = mybir.dt.float16
AF = mybir.ActivationFunctionType
ALU = mybir.AluOpType

N_CORES = 8
# B is overridable for cheap simulator runs (BASS_ATTN_B=1 -> 8 batches total).
B = int(os.environ.get("BASS_ATTN_B", "4"))  # batches per core
C = 256          # channels
NPIX = 1024      # pixels (32*32)
H = 8            # heads
D = 64           # head dim
HID = H * D      # 512
NT = NPIX // 128  # 8 n-tiles
CT = C // 128     # 2 c-tiles
QT = HID // 128   # 4 q-tiles
SCALE = D ** -0.5
BN_EPS = 1e-5
# Zq-broadcast matmul uses this instead of 1.0 so reciprocal(Zqb) directly
# yields SCALE / (n * Zq), folding the softmax scale and the v/n factor.
MASKVAL = NPIX / SCALE
N_WARM_MM = 24   # junk matmuls at t=0 keep PE busy so HAM un-throttles early


DEBUG = os.environ.get("BASS_ATTN_DEBUG") == "1"


def _emit(tc, x, wqkv, wout, gbuf, y, dbg=None):
    nc = tc.nc
    from contextlib import ExitStack
    ctx_stack = ExitStack()
    with ctx_stack:
        const = ctx_stack.enter_context(tc.tile_pool(name="const", bufs=1))
        kvsb = ctx_stack.enter_context(tc.tile_pool(name="kvsb", bufs=3))
        vxp = ctx_stack.enter_context(tc.tile_pool(name="vxp", bufs=3))
        qpool = ctx_stack.enter_context(tc.tile_pool(name="qpool", bufs=3))
        rpool = ctx_stack.enter_context(tc.tile_pool(name="rpool", bufs=3))
        cpool = ctx_stack.enter_context(tc.tile_pool(name="cpool", bufs=5))
        opool = ctx_stack.enter_context(tc.tile_pool(name="opool", bufs=6))
        fpool = ctx_stack.enter_context(tc.tile_pool(name="fpool", bufs=2 * B))
        small = ctx_stack.enter_context(tc.tile_pool(name="small", bufs=8))
        stats_p = ctx_stack.enter_context(tc.tile_pool(name="statsp", bufs=1))
        # PSUM: pbig 2 x 2-bank tiles + pctx 4 x 1-bank tiles = 8 banks
        pbig = ctx_stack.enter_context(
            tc.tile_pool(name="pbig", bufs=2, space="PSUM"))
        pctx = ctx_stack.enter_context(
            tc.tile_pool(name="pctx", bufs=4, space="PSUM"))
        dpool = ctx_stack.enter_context(
            tc.tile_pool(name="dram", bufs=1, space="DRAM"))

        # ---- constants / warmup ----
        eps_sb = const.tile([128, 1], F32, name="eps")
        nc.vector.memset(eps_sb, BN_EPS)
        # load the Exp table set while input DMAs are in flight
        warm_ex = small.tile([1, 1], F32, name="warmex")
        nc.scalar.activation(out=warm_ex, in_=eps_sb[0:1, :], func=AF.Exp)

        bmask = const.tile([128, 128], FP16, name="bmask")
        nc.vector.memset(bmask, 0.0)
        nc.vector.memset(bmask[0:64, 0:64], MASKVAL)
        nc.vector.memset(bmask[64:128, 64:128], MASKVAL)

        # x batch 0 + kv weight columns first (they gate the first matmul)
        xin = [[None] * CT for _ in range(B)]
        wqkv_sb = [const.tile([128, 3 * HID], FP16, name=f"wqkv{kc}")
                   for kc in range(CT)]
        for kc in range(CT):
            xt = const.tile([128, NPIX], FP16, name=f"x0_{kc}")
            nc.sync.dma_start(out=xt, in_=x[0, 128 * kc:128 * (kc + 1), :])
            xin[0][kc] = xt
            nc.sync.dma_start(out=wqkv_sb[kc][:, HID:3 * HID],
                              in_=wqkv[128 * kc:128 * (kc + 1), HID:3 * HID])
        for kc in range(CT):
            nc.sync.dma_start(out=wqkv_sb[kc][:, 0:HID],
                              in_=wqkv[128 * kc:128 * (kc + 1), 0:HID])

        # junk matmuls: keep PE busy from ~t=0 so the HAM clock-gate opens
        # (K=8/8) before the first real matmul instead of ~10us in
        jw = pbig.tile([128, NPIX], F32, name="junk", tag="pb")
        for i in range(N_WARM_MM):
            nc.tensor.matmul(jw[:, 0:128], lhsT=bmask, rhs=bmask,
                             start=True, stop=True)

        # remaining weights / inputs on other queues
        wout_sb = const.tile([128, 4 * C], FP16, name="wout")
        nc.scalar.dma_start(out=wout_sb, in_=wout)
        for b in range(1, B):
            for kc in range(CT):
                xt = const.tile([128, NPIX], FP16, name=f"x{b}_{kc}")
                nc.scalar.dma_start(
                    out=xt, in_=x[b, 128 * kc:128 * (kc + 1), :])
                xin[b][kc] = xt
        gb_sb = const.tile([128, 4], F32, name="gb")
        nc.scalar.dma_start(out=gb_sb, in_=gbuf)

        # dummy collective issued up front: the first AllReduce pays a
        # ~24us+ ncfw rendezvous; running it early overlaps that with compute
        no_cc = os.environ.get("BASS_ATTN_NO_CC") == "1"
        if not no_cc:
            wrm_i = dpool.tile([128, 1], F32, name="wrm_i")
            wrm_o = dpool.tile([128, 1], F32, name="wrm_o")
            nc.sync.dma_start(out=wrm_i, in_=eps_sb)
            nc.gpsimd.collective_compute(
                "AllReduce", ALU.add,
                replica_groups=[list(range(N_CORES))],
                ins=[wrm_i.opt()], outs=[wrm_o.opt()])

        stats_sb = [stats_p.tile([128, 2 * B, 6], F32, name=f"stats{m}")
                    for m in range(CT)]
        final_sb = [[None] * CT for _ in range(B)]

        # block-diagonal stationary tiles for the out matmuls: off-diagonal
        # 64x64 blocks stay zero forever, so memset each rotating buffer once
        N_BD = 5
        bd_bufs = [cpool.tile([128, 128], FP16, name=f"bd{i}")
                   for i in range(N_BD)]
        for t_ in bd_bufs:
            nc.vector.memset(t_, 0.0)

        # deferred final-projection emitter: interleaved into the next
        # batch's kv phase so the PE never drains at batch boundaries
        def emit_final(b):
            for m in range(CT):
                fs = fpool.tile([128, NPIX], FP16, name="final")
                for nch in range(2):
                    fh = pctx.tile([128, 512], F32, name="fp", tag="pc")
                    for k4 in range(QT):
                        nc.tensor.matmul(
                            fh,
                            lhsT=wout_sb[:, k4 * C + 128 * m:
                                         k4 * C + 128 * (m + 1)],
                            rhs=os_sb[b][k4][:, 512 * nch:512 * (nch + 1)],
                            start=(k4 == 0), stop=(k4 == QT - 1))
                    fsh = fs[:, 512 * nch:512 * (nch + 1)]
                    nc.scalar.copy(fsh, fh)
                    nc.vector.bn_stats(
                        out=stats_sb[m][:, 2 * b + nch, :], in_=fsh)
                final_sb[b][m] = fs

        os_sb = [[None] * QT for _ in range(B)]

        for b in range(B):
            xc = xin[b]

            # ---- KV projection + context accumulation over n-chunks ----
            ctxu = [pctx.tile([128, 512], F32, name="ctxu", tag="pc")
                    for _ in range(4)]
            kvp = [None] * NT

            def emit_kv_mm(t):
                # kc outer: consecutive matmul pairs share the stationary
                # operand (the x chunk); k and v halves are separate
                # accumulation groups in separate banks
                hp = pbig.tile([128, NPIX], F32, name="kvp", tag="pb")
                for kc in range(CT):
                    for nch in range(2):
                        nc.tensor.matmul(
                            hp[:, 512 * nch:512 * (nch + 1)],
                            lhsT=xc[kc][:, 128 * t:128 * (t + 1)],
                            rhs=wqkv_sb[kc][:, HID + 512 * nch:
                                            HID + 512 * (nch + 1)],
                            start=(kc == 0), stop=(kc == CT - 1))
                kvp[t] = hp

            def emit_kv_drain(t):
                ek = kvsb.tile([128, HID], FP16, name="expk")
                nc.scalar.activation(out=ek, in_=kvp[t][:, 0:512],
                                     func=AF.Exp)
                vx = vxp.tile([128, H, D + 1], FP16, name="vx")
                nc.vector.memset(vx[:, :, D:D + 1], 1.0)
                nc.vector.tensor_copy(
                    vx[:, :, 0:D],
                    kvp[t][:, 512:1024].rearrange("p (h e) -> p h e", h=H))
                return ek, vx

            def emit_ctx_mm(t, ek, vx):
                # one matmul per head-pair: out [128, 130] holds
                # (d=h0 rows x [e_h0 | Zk_h0] cols 0:65) and
                # (d=h1 rows x [e_h1 | Zk_h1] cols 65:130); the cross-head
                # blocks are computed but never read.
                for pr in range(4):
                    nc.tensor.matmul(
                        ctxu[pr][:, 0:2 * (D + 1)],
                        lhsT=ek[:, 128 * pr:128 * (pr + 1)],
                        rhs=vx[:, 2 * pr:2 * (pr + 1), :],
                        start=(t == 0), stop=(t == NT - 1))

            # software-pipelined: kv matmuls for t+1 are emitted before the
            # ctx matmuls for t, so the PE queue never stalls on the drains.
            # The previous batch's final projection is interleaved here too.
            drains = [None] * NT
            emit_kv_mm(0)
            if b > 0:
                emit_final(b - 1)

            for t in range(NT):
                if t + 1 < NT:
                    emit_kv_mm(t + 1)
                drains[t] = emit_kv_drain(t)
                emit_ctx_mm(t, *drains[t])

            # ---- context normalization (per-partition Zk) into the
            # block-diagonal out-matmul stationary ----
            cs = []
            for pr in range(4):
                if DEBUG and b == 0:
                    ct = small.tile([128, 2 * (D + 1)], F32, name="dbgct")
                    nc.vector.tensor_copy(ct, ctxu[pr][:, 0:2 * (D + 1)])
                    nc.sync.dma_start(out=dbg["ctx"][pr], in_=ct)
                rz = small.tile([128, 1], F32, name="rz")
                # the cross-head block's ones column at col 64 carries Zk for
                # BOTH heads (rows 0:64 = h0, rows 64:128 = h1)
                nc.vector.reciprocal_approx_fast(
                    out=rz, in_=ctxu[pr][:, D:D + 1])
                bd = bd_bufs[(b * 4 + pr) % N_BD]
                nc.vector.tensor_scalar_mul(
                    bd[0:64, 0:64], in0=ctxu[pr][0:64, 0:D],
                    scalar1=rz[0:64, :])
                nc.vector.tensor_scalar_mul(
                    bd[64:128, 64:128], in0=ctxu[pr][64:128, D + 1:2 * D + 1],
                    scalar1=rz[64:128, :])
                if DEBUG and b == 0:
                    nc.sync.dma_start(out=dbg["rz"][pr], in_=rz)
                cs.append(bd)

            # ---- Q projection, exp, Zq block-broadcast, reciprocal ----
            expq, recipb = [None] * QT, [None] * QT

            def emit_q_mm(t):
                qp = pbig.tile([128, NPIX], F32, name="qp", tag="pb")
                for kc in range(CT):
                    for nch in range(2):
                        nc.tensor.matmul(
                            qp[:, 512 * nch:512 * (nch + 1)],
                            lhsT=wqkv_sb[kc][:, 128 * t:128 * (t + 1)],
                            rhs=xc[kc][:, 512 * nch:512 * (nch + 1)],
                            start=(kc == 0), stop=(kc == CT - 1))
                return qp

            def emit_q_drain(t, qp):
                eq = qpool.tile([128, NPIX], FP16, name="expq")
                nc.scalar.activation(out=eq, in_=qp, func=AF.Exp)
                expq[t] = eq

            def emit_zq(t):
                rb = rpool.tile([128, NPIX], F32, name="recipb")
                for nch in range(2):
                    zp = pctx.tile([128, 512], F32, name="zq", tag="pc")
                    nc.tensor.matmul(
                        zp, lhsT=bmask,
                        rhs=expq[t][:, 512 * nch:512 * (nch + 1)],
                        start=True, stop=True)
                    nc.vector.reciprocal_approx_fast(
                        out=rb[:, 512 * nch:512 * (nch + 1)], in_=zp)
                recipb[t] = rb

            def emit_out(t):
                oh = pbig.tile([128, NPIX], F32, name="ou", tag="pb")
                for nch in range(2):
                    # block-diagonal stationary: one full-width matmul
                    # covers both heads of the pair
                    nc.tensor.matmul(
                        oh[:, 512 * nch:512 * (nch + 1)],
                        lhsT=cs[t],
                        rhs=expq[t][:, 512 * nch:512 * (nch + 1)],
                        start=True, stop=True)
                os_ = opool.tile([128, NPIX], FP16, name="outsb")
                nc.vector.tensor_mul(os_, oh, recipb[t])
                os_sb[b][t] = os_

            # pipeline: qmm(t+1) is emitted before zq(t) (which waits on the
            # ACT exp of tile t), and out(t) trails by one more stage
            qps = [None] * QT
            qps[0] = emit_q_mm(0)
            for t in range(QT):
                emit_q_drain(t, qps[t])
                if t + 1 < QT:
                    qps[t + 1] = emit_q_mm(t + 1)
                emit_zq(t)
                if t >= 1:
                    emit_out(t - 1)
            emit_out(QT - 1)

            if DEBUG and b == 0:
                nc.sync.dma_start(out=dbg["expk"], in_=drains[0][0])
                nc.sync.dma_start(out=dbg["vx"], in_=drains[0][1])
                for pr in range(4):
                    nc.sync.dma_start(out=dbg["bd"][pr], in_=cs[pr])
                    nc.sync.dma_start(out=dbg["os"][pr], in_=os_sb[b][pr])
                    nc.sync.dma_start(out=dbg["rb"][pr], in_=recipb[pr])
                    nc.sync.dma_start(out=dbg["eq"][pr], in_=expq[pr])


        emit_final(B - 1)

        # ---- batch-norm: aggregate, exchange stats via raw remote DMA ----
        # switch the ACT table to the sqrt set while PE still runs the last
        # final-proj matmuls, so the tail's Sqrt doesn't pay the ~2.7us load
        warm_sq = small.tile([1, 1], F32, name="warmsq")
        nc.scalar.activation(out=warm_sq, in_=eps_sb[0:1, :], func=AF.Sqrt)
        pk4 = small.tile([128, 2 * CT], F32, name="pk4")
        for m in range(CT):
            mv = small.tile([128, 2], F32, name="mv")
            nc.vector.bn_aggr(out=mv, in_=stats_sb[m])
            pk = pk4[:, 2 * m:2 * (m + 1)]
            nc.vector.tensor_mul(pk[:, 1:2], mv[:, 0:1], mv[:, 0:1])
            nc.vector.tensor_add(pk[:, 1:2], pk[:, 1:2], mv[:, 1:2])
            nc.vector.tensor_copy(pk[:, 0:1], mv[:, 0:1])
            nc.vector.tensor_scalar_mul(pk, in0=pk, scalar1=1.0 / N_CORES)
        if no_cc:
            gst = small.tile([128, 2 * CT], F32, name="gst")
            nc.vector.tensor_copy(gst, pk4)
            # in-Tile tail (simulator path only)
            g2v = gst.rearrange("p (m k) -> p m k", k=2)
            gmean2 = g2v[:, :, 0]
            gex22 = g2v[:, :, 1]
            var2 = small.tile([128, 2], F32, name="var2")
            nc.vector.tensor_mul(var2, gmean2, gmean2)
            nc.vector.tensor_sub(var2, gex22, var2)
            std2 = small.tile([128, 2], F32, name="std2")
            nc.scalar.activation(out=std2, in_=var2, func=AF.Sqrt, bias=eps_sb)
            rsg2 = small.tile([128, 2], F32, name="rsg2")
            nc.vector.reciprocal_approx_fast(out=rsg2, in_=std2)
            nc.vector.tensor_mul(rsg2, rsg2, gb_sb[:, 0:2])
            sh2 = small.tile([128, 2], F32, name="sh2")
            nc.vector.tensor_mul(sh2, gmean2, rsg2)
            nc.vector.tensor_sub(sh2, gb_sb[:, 2:4], sh2)
            for m in range(CT):
                for b in range(B):
                    fs = final_sb[b][m]
                    nc.vector.tensor_scalar(
                        out=fs, in0=fs, scalar1=rsg2[:, m:m + 1],
                        scalar2=sh2[:, m:m + 1], op0=ALU.mult, op1=ALU.add)
                    nc.sync.dma_start(
                        out=y[b, 128 * m:128 * (m + 1), :], in_=fs)
            return None

        ccin = dpool.tile([128, 2 * CT], F32, name="ccin")
        ccout = dpool.tile([128, 2 * CT], F32, name="ccout")
        nc.sync.dma_start(out=ccin, in_=pk4)
        nc.gpsimd.collective_compute(
            "AllReduce", ALU.add,
            replica_groups=[list(range(N_CORES))],
            ins=[ccin.opt()], outs=[ccout.opt()])
        gst = small.tile([128, 2 * CT], F32, name="gst")
        nc.sync.dma_start(out=gst, in_=ccout)
        g2v = gst.rearrange("p (m k) -> p m k", k=2)
        gmean2 = g2v[:, :, 0]
        gex22 = g2v[:, :, 1]
        var2 = small.tile([128, 2], F32, name="var2")
        nc.vector.tensor_mul(var2, gmean2, gmean2)
        nc.vector.tensor_sub(var2, gex22, var2)
        std2 = small.tile([128, 2], F32, name="std2")
        nc.scalar.activation(out=std2, in_=var2, func=AF.Sqrt, bias=eps_sb)
        rsg2 = small.tile([128, 2], F32, name="rsg2")
        nc.vector.reciprocal_approx_fast(out=rsg2, in_=std2)
        nc.vector.tensor_mul(rsg2, rsg2, gb_sb[:, 0:2])
        sh2 = small.tile([128, 2], F32, name="sh2")
        nc.vector.tensor_mul(sh2, gmean2, rsg2)
        nc.vector.tensor_sub(sh2, gb_sb[:, 2:4], sh2)
        dmaq = [nc.sync, nc.gpsimd, nc.scalar, nc.gpsimd]
        for m in range(CT):
            rsg = rsg2[:, m:m + 1]
            sh = sh2[:, m:m + 1]
            for b in range(B):
                fs = final_sb[b][m]
                if b == 1:
                    nc.scalar.activation(
                        out=fs, in_=fs, func=AF.Identity,
                        bias=sh, scale=rsg)
                else:
                    nc.vector.tensor_scalar(
                        out=fs, in0=fs, scalar1=rsg, scalar2=sh,
                        op0=ALU.mult, op1=ALU.add)
                dmaq[(2 * m + b) % 4].dma_start(
                    out=y[b, 128 * m:128 * (m + 1), :], in_=fs)
        return None


def _c(ap):
    """Concretize a (possibly tile-backed) AP for raw post-Tile emission."""
    if not hasattr(ap, "tensor"):
        ap = ap[:]
    t = ap.tensor
    if hasattr(t, "concrete_tensor"):
        ap.tensor = t.concrete_tensor()
    return ap


def _emit_epilogue(nc, h, y):
    """Raw post-Tile tail: wait for peer stats, reduce, normalize, store."""
    V, S, SY = nc.vector, nc.scalar, nc.sync
    s1 = nc.alloc_semaphore("ep_s1")
    s2 = nc.alloc_semaphore("ep_s2")
    s3 = nc.alloc_semaphore("ep_s3")
    rsem = h["rsem"]
    rbuf = _c(h["rbuf"])
    r4, r2, gst = _c(h["r4"]), _c(h["r2"]), _c(h["gst"])
    var2, std2 = _c(h["var2"]), _c(h["std2"])
    rsg2, sh2 = _c(h["rsg2"]), _c(h["sh2"])
    eps, gb = _c(h["eps"]), _c(h["gb"])
    fsc = [[_c(h["final_sb"][b][m]) for m in range(CT)] for b in range(B)]
    npeer = int(os.environ.get("BASS_ATTN_NPEER", 2 * (N_CORES - 1)))
    V.wait_ge(rsem, npeer)
    V.sem_inc(rsem, -npeer)
    V.tensor_add(r4, rbuf[:, 0:4, :], rbuf[:, 4:8, :])
    V.tensor_add(r2, r4[:, 0:2, :], r4[:, 2:4, :])
    V.tensor_add(gst, r2[:, 0, :], r2[:, 1, :])
    g2v = gst.rearrange("p (m k) -> p m k", k=2)
    gmean2 = g2v[:, :, 0]
    gex22 = g2v[:, :, 1]
    V.tensor_mul(var2, gmean2, gmean2)
    V.tensor_sub(var2, gex22, var2).then_inc(s1, 1)
    S.wait_ge(s1, 1)
    S.sem_inc(s1, -1)
    S.activation(out=std2, in_=var2, func=AF.Sqrt, bias=eps).then_inc(s2, 1)
    V.wait_ge(s2, 1)
    V.sem_inc(s2, -1)
    V.reciprocal_approx_fast(out=rsg2, in_=std2)
    V.tensor_mul(rsg2, rsg2, gb[:, 0:2])
    V.tensor_mul(sh2, gmean2, rsg2)
    V.tensor_sub(sh2, gb[:, 2:4], sh2)
    s4 = nc.alloc_semaphore("ep_s4")
    dsem = nc.alloc_semaphore("ep_dsem")
    tiles = [(m, b) for m in range(CT) for b in range(B)]
    for i, (m, b) in enumerate(tiles):
        fs = fsc[b][m]
        # first 4 tiles signal sync's sem, last 4 scalar's; each DMA
        # engine waits on and consumes only its own sem
        V.tensor_scalar(out=fs, in0=fs, scalar1=rsg2[:, m:m + 1],
                        scalar2=sh2[:, m:m + 1],
                        op0=ALU.mult, op1=ALU.add).then_inc(
                            s3 if i < 4 else s4, 1)
    for i, (m, b) in enumerate(tiles):
        eng, sem, tgt = (SY, s3, i + 1) if i < 4 else (S, s4, i - 3)
        eng.wait_ge(sem, tgt)
        eng.dma_start(out=y[b, 128 * m:128 * (m + 1), :],
                      in_=fsc[b][m]).then_inc(dsem, 16)
    SY.sem_inc(s3, -4)
    S.sem_inc(s4, -4)
    # all 8 output DMAs complete before the program ends
    SY.wait_ge(dsem, 16 * 2 * B * CT // 2)
    SY.sem_inc(dsem, -16 * 2 * B * CT // 2)


_CACHE = {}


def _build():
    if "nc" in _CACHE:
        return _CACHE["nc"]
    nc = bacc.Bacc("TRN2", target_bir_lowering=False, debug=False,
                   enable_asserts=True, num_devices=N_CORES)
    x = nc.dram_tensor("x", [B, C, NPIX], FP16, kind="ExternalInput").ap()
    wqkv = nc.dram_tensor("wqkvT", [C, 3 * HID], FP16,
                          kind="ExternalInput").ap()
    wout = nc.dram_tensor("woutT", [128, 4 * C], FP16,
                          kind="ExternalInput").ap()
    gbuf = nc.dram_tensor("gbuf", [128, 4], F32, kind="ExternalInput").ap()
    y = nc.dram_tensor("y", [B, C, NPIX], FP16, kind="ExternalOutput").ap()
    dbg = None
    if DEBUG:
        dbg = {
            "expk": nc.dram_tensor("d_expk", [128, HID], FP16,
                                   kind="ExternalOutput").ap(),
            "vx": nc.dram_tensor("d_vx", [128, H, D + 1], FP16,
                                 kind="ExternalOutput").ap(),
            "bd": [nc.dram_tensor(f"d_bd{p}", [128, 128], FP16,
                                  kind="ExternalOutput").ap() for p in range(4)],
            "os": [nc.dram_tensor(f"d_os{p}", [128, NPIX], FP16,
                                  kind="ExternalOutput").ap() for p in range(4)],
            "rb": [nc.dram_tensor(f"d_rb{p}", [128, NPIX], F32,
                                  kind="ExternalOutput").ap() for p in range(4)],
            "eq": [nc.dram_tensor(f"d_eq{p}", [128, NPIX], FP16,
                                  kind="ExternalOutput").ap() for p in range(4)],
            "ctx": [nc.dram_tensor(f"d_ctx{p}", [128, 2 * (D + 1)], F32,
                                   kind="ExternalOutput").ap() for p in range(4)],
            "rz": [nc.dram_tensor(f"d_rz{p}", [128, 1], F32,
                                  kind="ExternalOutput").ap() for p in range(4)],
        }
    with tile.TileContext(nc) as tc:
        h = _emit(tc, x, wqkv, wout, gbuf, y, dbg)
    if h is not None:
        _emit_epilogue(nc, h, y)
    nc.compile()
    _CACHE["nc"] = nc
    return nc


def kernel(x, w_qkv, w_out, b_out, gamma, beta, _trace=False):
    x = np.asarray(x, dtype=np.float32)
    wqkvT = np.ascontiguousarray(np.asarray(w_qkv, np.float16).T)   # [256, 1536]
    woutT = np.asarray(w_out, np.float16).T                         # [512, 256]
    # [128, 4*256]: woutT[k4*128+p, c] -> [p, k4*256+c]
    woutp = np.ascontiguousarray(
        woutT.reshape(4, 128, C).transpose(1, 0, 2).reshape(128, 4 * C))
    # gbuf cols: gamma m=0, gamma m=1, beta m=0, beta m=1
    g2 = np.asarray(gamma, np.float32).reshape(CT, 128).T
    b2 = np.asarray(beta, np.float32).reshape(CT, 128).T
    gbuf = np.ascontiguousarray(np.concatenate([g2, b2], axis=1))
    # b_out is intentionally unused: BatchNorm's mean subtraction cancels any
    # per-channel constant added before it, exactly.

    btot, c, hh, ww = x.shape
    assert (btot, c, hh * ww) == (B * N_CORES, C, NPIX)
    xf = x.reshape(btot, C, NPIX)

    nc = _build()
    in_maps = []
    for core in range(N_CORES):
        in_maps.append({
            "x": np.ascontiguousarray(
                xf[B * core:B * (core + 1)]).astype(np.float16),
            "wqkvT": wqkvT,
            "woutT": woutp,
            "gbuf": gbuf,
        })
    res = run_bass_kernel_spmd(nc, in_maps, core_ids=list(range(N_CORES)),
                               trace=_trace)
    y = np.concatenate([np.asarray(res.results[core]["y"], np.float32)
                        for core in range(N_CORES)], axis=0)
    out = y.reshape(btot, C, hh, ww)
    if _trace:
        kernel.last_result = res
    return out


# revision 40
# speedup vs baseline: 1.2573x; 1.0886x over previous
"""Trainium2 Bass kernel for nn_AttentionBlock (linear attention + BatchNorm).

Math (per batch, c=256 channels, n=1024 pixels, 8 heads x 64 dims):
  qkv = w_qkv @ x                      [1536, n]
  q   = softmax(q, axis=d) * d^-0.5    (per head, over the 64 head-dims)
  k   = softmax(k, axis=n)             (per head-dim, over pixels)
  ctx = k @ (v/n)^T                    [d, e] per head
  out = ctx^T @ q                      [e, n] per head
  y   = BatchNorm(w_out @ out + b_out) (batch stats over (b, n) per channel)

Sharding: data-parallel over batch across 8 cores (4 batches each); BN batch
stats are combined with a tiny AllReduce (2 floats per channel). b_out is
skipped: BatchNorm's mean subtraction cancels any per-channel constant exactly.

Device layouts (per batch):
  x      [c, n]           c on partitions (2 tiles, fp16)
  kv     [n, (k|v)]       per n-tile one 2-bank PSUM tile: k cols 0:512,
                          v cols 512:1024.  exp(k) -> expk fp16 (ACT);
                          v -> vx [128,8,65] fp16 with a ones column.
  ctx_h  [d, e+1]         contraction over n (8 chunks, PSUM accum, 4 banks);
                          col 64 = Zk (ones column) -> per-partition norm
  q      [(h d), n]       one 2-bank PSUM tile per q-tile; ONE exp -> expq
  Zq     [(h d), n]       block-mask matmul; recip folds SCALE and 1/n
  out_h  [e, n]           lhsT=cs, rhs=expq, quadrant-packed pairs of heads,
                          2-bank PSUM; normalized by recipb -> os fp16
  final  [c, n]           lhsT=w_out^T; fs fp16 resident; bn_stats per (b,m);
                          AllReduce of packed (mean, E[x^2]); normalize in
                          place; DMA out fp16.
"""

import os
import sys

import numpy as np

for _p in ("/opt/trn_rl_repo", "/root/.axon_site/_ro/trn_rl_repo"):
    if os.path.isdir(_p) and _p not in sys.path:
        sys.path.insert(0, _p)

import concourse.bacc as bacc
import concourse.tile as tile
from concourse import mybir
from concourse.bass_utils import run_bass_kernel_spmd

F32 = mybir.dt.float32
FP16 = mybir.dt.float16
AF = mybir.ActivationFunctionType
ALU = mybir.AluOpType

N_CORES = 8
# B is overridable for cheap simulator runs (BASS_ATTN_B=1 -> 8 batches total).
B = int(os.environ.get("BASS_ATTN_B", "4"))  # batches per core
C = 256          # channels
NPIX = 1024      # pixels (32*32)
H = 8            # heads
D = 64           # head dim
HID = H * D      # 512
NT = NPIX // 128  # 8 n-tiles
CT = C // 128     # 2 c-tiles
QT = HID // 128   # 4 q-tiles
SCALE = D ** -0.5
BN_EPS = 1e-5
# Zq-broadcast matmul uses this instead of 1.0 so reciprocal(Zqb) directly
# yields SCALE / (n * Zq), folding the softmax scale and the v/n factor.
MASKVAL = NPIX / SCALE
N_WARM_MM = 24   # junk matmuls at t=0 keep PE busy so HAM un-throttles early


DEBUG = os.environ.get("BASS_ATTN_DEBUG") == "1"


def _emit(tc, x, wqkv, wout, gbuf, y, dbg=None):
    nc = tc.nc
    from contextlib import ExitStack
    ctx_stack = ExitStack()
    with ctx_stack:
        const = ctx_stack.enter_context(tc.tile_pool(name="const", bufs=1))
        kvsb = ctx_stack.enter_context(tc.tile_pool(name="kvsb", bufs=3))
        vxp = ctx_stack.enter_context(tc.tile_pool(name="vxp", bufs=3))
        qpool = ctx_stack.enter_context(tc.tile_pool(name="qpool", bufs=3))
        rpool = ctx_stack.enter_context(tc.tile_pool(name="rpool", bufs=3))
        cpool = ctx_stack.enter_context(tc.tile_pool(name="cpool", bufs=5))
        opool = ctx_stack.enter_context(tc.tile_pool(name="opool", bufs=6))
        fpool = ctx_stack.enter_context(tc.tile_pool(name="fpool", bufs=2 * B))
        small = ctx_stack.enter_context(tc.tile_pool(name="small", bufs=8))
        stats_p = ctx_stack.enter_context(tc.tile_pool(name="statsp", bufs=1))
        # PSUM: pbig 2 x 2-bank tiles + pctx 4 x 1-bank tiles = 8 banks
        pbig = ctx_stack.enter_context(
            tc.tile_pool(name="pbig", bufs=2, space="PSUM"))
        pctx = ctx_stack.enter_context(
            tc.tile_pool(name="pctx", bufs=4, space="PSUM"))
        dpool = ctx_stack.enter_context(
            tc.tile_pool(name="dram", bufs=1, space="DRAM"))

        # ---- constants / warmup ----
        eps_sb = const.tile([128, 1], F32, name="eps")
        nc.vector.memset(eps_sb, BN_EPS)
        # load the Exp table set while input DMAs are in flight
        warm_ex = small.tile([1, 1], F32, name="warmex")
        nc.scalar.activation(out=warm_ex, in_=eps_sb[0:1, :], func=AF.Exp)

        bmask = const.tile([128, 128], FP16, name="bmask")
        nc.vector.memset(bmask, 0.0)
        nc.vector.memset(bmask[0:64, 0:64], MASKVAL)
        nc.vector.memset(bmask[64:128, 64:128], MASKVAL)

        # x batch 0 + kv weight columns first (they gate the first matmul)
        xin = [[None] * CT for _ in range(B)]
        wqkv_sb = [const.tile([128, 3 * HID], FP16, name=f"wqkv{kc}")
                   for kc in range(CT)]
        for kc in range(CT):
            xt = const.tile([128, NPIX], FP16, name=f"x0_{kc}")
            nc.sync.dma_start(out=xt, in_=x[0, 128 * kc:128 * (kc + 1), :])
            xin[0][kc] = xt
            nc.sync.dma_start(out=wqkv_sb[kc][:, HID:3 * HID],
                              in_=wqkv[128 * kc:128 * (kc + 1), HID:3 * HID])
        for kc in range(CT):
            nc.sync.dma_start(out=wqkv_sb[kc][:, 0:HID],
                              in_=wqkv[128 * kc:128 * (kc + 1), 0:HID])

        # junk matmuls: keep PE busy from ~t=0 so the HAM clock-gate opens
        # (K=8/8) before the first real matmul instead of ~10us in
        jw = pbig.tile([128, NPIX], F32, name="junk", tag="pb")
        for i in range(N_WARM_MM):
            nc.tensor.matmul(jw[:, 0:128], lhsT=bmask, rhs=bmask,
                             start=True, stop=True)

        # remaining weights / inputs on other queues
        wout_sb = const.tile([128, 4 * C], FP16, name="wout")
        nc.scalar.dma_start(out=wout_sb, in_=wout)
        for b in range(1, B):
            for kc in range(CT):
                xt = const.tile([128, NPIX], FP16, name=f"x{b}_{kc}")
                nc.scalar.dma_start(
                    out=xt, in_=x[b, 128 * kc:128 * (kc + 1), :])
                xin[b][kc] = xt
        gb_sb = const.tile([128, 4], F32, name="gb")
        nc.scalar.dma_start(out=gb_sb, in_=gbuf)

        # dummy collective issued up front: the first AllReduce pays a
        # ~24us+ ncfw rendezvous; running it early overlaps that with compute
        no_cc = os.environ.get("BASS_ATTN_NO_CC") == "1"
        if not no_cc:
            wrm_i = dpool.tile([128, 1], F32, name="wrm_i")
            wrm_o = dpool.tile([128, 1], F32, name="wrm_o")
            nc.sync.dma_start(out=wrm_i, in_=eps_sb)
            nc.gpsimd.collective_compute(
                "AllReduce", ALU.add,
                replica_groups=[list(range(N_CORES))],
                ins=[wrm_i.opt()], outs=[wrm_o.opt()])

        stats_sb = [stats_p.tile([128, 2 * B, 6], F32, name=f"stats{m}")
                    for m in range(CT)]
        final_sb = [[None] * CT for _ in range(B)]

        # block-diagonal stationary tiles for the out matmuls: off-diagonal
        # 64x64 blocks stay zero forever, so memset each rotating buffer once
        N_BD = 5
        bd_bufs = [cpool.tile([128, 128], FP16, name=f"bd{i}")
                   for i in range(N_BD)]
        for t_ in bd_bufs:
            nc.vector.memset(t_, 0.0)

        # deferred final-projection emitter: interleaved into the next
        # batch's kv phase so the PE never drains at batch boundaries
        def emit_final(b):
            for m in range(CT):
                fs = fpool.tile([128, NPIX], FP16, name="final")
                for nch in range(2):
                    fh = pctx.tile([128, 512], F32, name="fp", tag="pc")
                    for k4 in range(QT):
                        nc.tensor.matmul(
                            fh,
                            lhsT=wout_sb[:, k4 * C + 128 * m:
                                         k4 * C + 128 * (m + 1)],
                            rhs=os_sb[b][k4][:, 512 * nch:512 * (nch + 1)],
                            start=(k4 == 0), stop=(k4 == QT - 1))
                    fsh = fs[:, 512 * nch:512 * (nch + 1)]
                    nc.scalar.copy(fsh, fh)
                    nc.vector.bn_stats(
                        out=stats_sb[m][:, 2 * b + nch, :], in_=fsh)
                final_sb[b][m] = fs

        os_sb = [[None] * QT for _ in range(B)]

        for b in range(B):
            xc = xin[b]

            # ---- KV projection + context accumulation over n-chunks ----
            ctxu = [pctx.tile([128, 512], F32, name="ctxu", tag="pc")
                    for _ in range(4)]
            kvp = [None] * NT

            def emit_kv_mm(t):
                # kc outer: consecutive matmul pairs share the stationary
                # operand (the x chunk); k and v halves are separate
                # accumulation groups in separate banks
                hp = pbig.tile([128, NPIX], F32, name="kvp", tag="pb")
                for kc in range(CT):
                    for nch in range(2):
                        nc.tensor.matmul(
                            hp[:, 512 * nch:512 * (nch + 1)],
                            lhsT=xc[kc][:, 128 * t:128 * (t + 1)],
                            rhs=wqkv_sb[kc][:, HID + 512 * nch:
                                            HID + 512 * (nch + 1)],
                            start=(kc == 0), stop=(kc == CT - 1))
                kvp[t] = hp

            def emit_kv_drain(t):
                ek = kvsb.tile([128, HID], FP16, name="expk")
                nc.scalar.activation(out=ek, in_=kvp[t][:, 0:512],
                                     func=AF.Exp)
                vx = vxp.tile([128, H, D + 1], FP16, name="vx")
                nc.vector.memset(vx[:, :, D:D + 1], 1.0)
                nc.vector.tensor_copy(
                    vx[:, :, 0:D],
                    kvp[t][:, 512:1024].rearrange("p (h e) -> p h e", h=H))
                return ek, vx

            def emit_ctx_mm(t, ek, vx):
                # one matmul per head-pair: out [128, 130] holds
                # (d=h0 rows x [e_h0 | Zk_h0] cols 0:65) and
                # (d=h1 rows x [e_h1 | Zk_h1] cols 65:130); the cross-head
                # blocks are computed but never read.
                for pr in range(4):
                    nc.tensor.matmul(
                        ctxu[pr][:, 0:2 * (D + 1)],
                        lhsT=ek[:, 128 * pr:128 * (pr + 1)],
                        rhs=vx[:, 2 * pr:2 * (pr + 1), :],
                        start=(t == 0), stop=(t == NT - 1))

            # software-pipelined: kv matmuls for t+1 are emitted before the
            # ctx matmuls for t, so the PE queue never stalls on the drains.
            # The previous batch's final projection is interleaved here too.
            drains = [None] * NT
            emit_kv_mm(0)
            if b > 0:
                emit_final(b - 1)

            for t in range(NT):
                if t + 1 < NT:
                    emit_kv_mm(t + 1)
                drains[t] = emit_kv_drain(t)
                emit_ctx_mm(t, *drains[t])

            # ---- context normalization (per-partition Zk) into the
            # block-diagonal out-matmul stationary ----
            cs = []
            for pr in range(4):
                if DEBUG and b == 0:
                    ct = small.tile([128, 2 * (D + 1)], F32, name="dbgct")
                    nc.vector.tensor_copy(ct, ctxu[pr][:, 0:2 * (D + 1)])
                    nc.sync.dma_start(out=dbg["ctx"][pr], in_=ct)
                rz = small.tile([128, 1], F32, name="rz")
                # the cross-head block's ones column at col 64 carries Zk for
                # BOTH heads (rows 0:64 = h0, rows 64:128 = h1)
                nc.vector.reciprocal_approx_fast(
                    out=rz, in_=ctxu[pr][:, D:D + 1])
                bd = bd_bufs[(b * 4 + pr) % N_BD]
                nc.vector.tensor_scalar_mul(
                    bd[0:64, 0:64], in0=ctxu[pr][0:64, 0:D],
                    scalar1=rz[0:64, :])
                nc.vector.tensor_scalar_mul(
                    bd[64:128, 64:128], in0=ctxu[pr][64:128, D + 1:2 * D + 1],
                    scalar1=rz[64:128, :])
                if DEBUG and b == 0:
                    nc.sync.dma_start(out=dbg["rz"][pr], in_=rz)
                cs.append(bd)

            # ---- Q projection, exp, Zq block-broadcast, reciprocal ----
            expq, recipb = [None] * QT, [None] * QT

            def emit_q_mm(t):
                qp = pbig.tile([128, NPIX], F32, name="qp", tag="pb")
                for kc in range(CT):
                    for nch in range(2):
                        nc.tensor.matmul(
                            qp[:, 512 * nch:512 * (nch + 1)],
                            lhsT=wqkv_sb[kc][:, 128 * t:128 * (t + 1)],
                            rhs=xc[kc][:, 512 * nch:512 * (nch + 1)],
                            start=(kc == 0), stop=(kc == CT - 1))
                return qp

            def emit_q_drain(t, qp):
                eq = qpool.tile([128, NPIX], FP16, name="expq")
                nc.scalar.activation(out=eq, in_=qp, func=AF.Exp)
                expq[t] = eq

            def emit_zq(t):
                rb = rpool.tile([128, NPIX], F32, name="recipb")
                for nch in range(2):
                    zp = pctx.tile([128, 512], F32, name="zq", tag="pc")
                    nc.tensor.matmul(
                        zp, lhsT=bmask,
                        rhs=expq[t][:, 512 * nch:512 * (nch + 1)],
                        start=True, stop=True)
                    nc.vector.reciprocal_approx_fast(
                        out=rb[:, 512 * nch:512 * (nch + 1)], in_=zp)
                recipb[t] = rb

            def emit_out(t):
                oh = pbig.tile([128, NPIX], F32, name="ou", tag="pb")
                for nch in range(2):
                    # block-diagonal stationary: one full-width matmul
                    # covers both heads of the pair
                    nc.tensor.matmul(
                        oh[:, 512 * nch:512 * (nch + 1)],
                        lhsT=cs[t],
                        rhs=expq[t][:, 512 * nch:512 * (nch + 1)],
                        start=True, stop=True)
                os_ = opool.tile([128, NPIX], FP16, name="outsb")
                nc.vector.tensor_mul(os_, oh, recipb[t])
                os_sb[b][t] = os_

            # pipeline: qmm(t+1) is emitted before zq(t) (which waits on the
            # ACT exp of tile t), and out(t) trails by one more stage
            qps = [None] * QT
            qps[0] = emit_q_mm(0)
            for t in range(QT):
                emit_q_drain(t, qps[t])
                if t + 1 < QT:
                    qps[t + 1] = emit_q_mm(t + 1)
                emit_zq(t)
                if t >= 1:
                    emit_out(t - 1)
            emit_out(QT - 1)

            if DEBUG and b == 0:
                nc.sync.dma_start(out=dbg["expk"], in_=drains[0][0])
                nc.sync.dma_start(out=dbg["vx"], in_=drains[0][1])
                for pr in range(4):
                    nc.sync.dma_start(out=dbg["bd"][pr], in_=cs[pr])
                    nc.sync.dma_start(out=dbg["os"][pr], in_=os_sb[b][pr])
                    nc.sync.dma_start(out=dbg["rb"][pr], in_=recipb[pr])
                    nc.sync.dma_start(out=dbg["eq"][pr], in_=expq[pr])


        emit_final(B - 1)

        # ---- batch-norm: aggregate, exchange stats via raw remote DMA ----
        # switch the ACT table to the sqrt set while PE still runs the last
        # final-proj matmuls, so the tail's Sqrt doesn't pay the ~2.7us load
        warm_sq = small.tile([1, 1], F32, name="warmsq")
        nc.scalar.activation(out=warm_sq, in_=eps_sb[0:1, :], func=AF.Sqrt)
        pk4 = small.tile([128, 2 * CT], F32, name="pk4")
        for m in range(CT):
            mv = small.tile([128, 2], F32, name="mv")
            nc.vector.bn_aggr(out=mv, in_=stats_sb[m])
            pk = pk4[:, 2 * m:2 * (m + 1)]
            nc.vector.tensor_mul(pk[:, 1:2], mv[:, 0:1], mv[:, 0:1])
            nc.vector.tensor_add(pk[:, 1:2], pk[:, 1:2], mv[:, 1:2])
            nc.vector.tensor_copy(pk[:, 0:1], mv[:, 0:1])
            nc.vector.tensor_scalar_mul(pk, in0=pk, scalar1=1.0 / N_CORES)
        if no_cc:
            gst = small.tile([128, 2 * CT], F32, name="gst")
            nc.vector.tensor_copy(gst, pk4)
            # in-Tile tail (simulator path only)
            g2v = gst.rearrange("p (m k) -> p m k", k=2)
            gmean2 = g2v[:, :, 0]
            gex22 = g2v[:, :, 1]
            var2 = small.tile([128, 2], F32, name="var2")
            nc.vector.tensor_mul(var2, gmean2, gmean2)
            nc.vector.tensor_sub(var2, gex22, var2)
            std2 = small.tile([128, 2], F32, name="std2")
            nc.scalar.activation(out=std2, in_=var2, func=AF.Sqrt, bias=eps_sb)
            rsg2 = small.tile([128, 2], F32, name="rsg2")
            nc.vector.reciprocal_approx_fast(out=rsg2, in_=std2)
            nc.vector.tensor_mul(rsg2, rsg2, gb_sb[:, 0:2])
            sh2 = small.tile([128, 2], F32, name="sh2")
            nc.vector.tensor_mul(sh2, gmean2, rsg2)
            nc.vector.tensor_sub(sh2, gb_sb[:, 2:4], sh2)
            for m in range(CT):
                for b in range(B):
                    fs = final_sb[b][m]
                    nc.vector.tensor_scalar(
                        out=fs, in0=fs, scalar1=rsg2[:, m:m + 1],
                        scalar2=sh2[:, m:m + 1], op0=ALU.mult, op1=ALU.add)
                    nc.sync.dma_start(
                        out=y[b, 128 * m:128 * (m + 1), :], in_=fs)
            return None

        ccin = dpool.tile([128, 2 * CT], F32, name="ccin")
        ccout = dpool.tile([128, 2 * CT], F32, name="ccout")
        nc.sync.dma_start(out=ccin, in_=pk4)
        nc.gpsimd.collective_compute(
            "AllReduce", ALU.add,
            replica_groups=[list(range(N_CORES))],
            ins=[ccin.opt()], outs=[ccout.opt()])
        gst = small.tile([128, 2 * CT], F32, name="gst")
        nc.sync.dma_start(out=gst, in_=ccout)
        g2v = gst.rearrange("p (m k) -> p m k", k=2)
        gmean2 = g2v[:, :, 0]
        gex22 = g2v[:, :, 1]
        var2 = small.tile([128, 2], F32, name="var2")
        nc.vector.tensor_mul(var2, gmean2, gmean2)
        nc.vector.tensor_sub(var2, gex22, var2)
        std2 = small.tile([128, 2], F32, name="std2")
        nc.scalar.activation(out=std2, in_=var2, func=AF.Sqrt, bias=eps_sb)
        rsg2 = small.tile([128, 2], F32, name="rsg2")
        nc.vector.reciprocal_approx_fast(out=rsg2, in_=std2)
        nc.vector.tensor_mul(rsg2, rsg2, gb_sb[:, 0:2])
        sh2 = small.tile([128, 2], F32, name="sh2")
        nc.vector.tensor_mul(sh2, gmean2, rsg2)
        nc.vector.tensor_sub(sh2, gb_sb[:, 2:4], sh2)
        dmaq = [nc.sync, nc.gpsimd, nc.scalar, nc.gpsimd]
        for m in range(CT):
            rsg = rsg2[:, m:m + 1]
            sh = sh2[:, m:m + 1]
            for b in range(B):
                fs = final_sb[b][m]
                if b == 1:
                    nc.scalar.activation(
                        out=fs, in_=fs, func=AF.Identity,
                        bias=sh, scale=rsg)
                else:
                    nc.vector.tensor_scalar(
                        out=fs, in0=fs, scalar1=rsg, scalar2=sh,
                        op0=ALU.mult, op1=ALU.add)
                dmaq[(2 * m + b) % 4].dma_start(
                    out=y[b, 128 * m:128 * (m + 1), :], in_=fs)
        return None


_CACHE = {}


def _build():
    if "nc" in _CACHE:
        return _CACHE["nc"]
    nc = bacc.Bacc("TRN2", target_bir_lowering=False, debug=False,
                   enable_asserts=True, num_devices=N_CORES)
    x = nc.dram_tensor("x", [B, C, NPIX], FP16, kind="ExternalInput").ap()
    wqkv = nc.dram_tensor("wqkvT", [C, 3 * HID], FP16,
                          kind="ExternalInput").ap()
    wout = nc.dram_tensor("woutT", [128, 4 * C], FP16,
                          kind="ExternalInput").ap()
    gbuf = nc.dram_tensor("gbuf", [128, 4], F32, kind="ExternalInput").ap()
    y = nc.dram_tensor("y", [B, C, NPIX], FP16, kind="ExternalOutput").ap()
    dbg = None
    if DEBUG:
        dbg = {
            "expk": nc.dram_tensor("d_expk", [128, HID], FP16,
                                   kind="ExternalOutput").ap(),
            "vx": nc.dram_tensor("d_vx", [128, H, D + 1], FP16,
                                 kind="ExternalOutput").ap(),
            "bd": [nc.dram_tensor(f"d_bd{p}", [128, 128], FP16,
                                  kind="ExternalOutput").ap() for p in range(4)],
            "os": [nc.dram_tensor(f"d_os{p}", [128, NPIX], FP16,
                                  kind="ExternalOutput").ap() for p in range(4)],
            "rb": [nc.dram_tensor(f"d_rb{p}", [128, NPIX], F32,
                                  kind="ExternalOutput").ap() for p in range(4)],
            "eq": [nc.dram_tensor(f"d_eq{p}", [128, NPIX], FP16,
                                  kind="ExternalOutput").ap() for p in range(4)],
            "ctx": [nc.dram_tensor(f"d_ctx{p}", [128, 2 * (D + 1)], F32,
                                   kind="ExternalOutput").ap() for p in range(4)],
            "rz": [nc.dram_tensor(f"d_rz{p}", [128, 1], F32,
                                  kind="ExternalOutput").ap() for p in range(4)],
        }
    with tile.TileContext(nc) as tc:
        _emit(tc, x, wqkv, wout, gbuf, y, dbg)
    nc.compile()
    _CACHE["nc"] = nc
    return nc


def kernel(x, w_qkv, w_out, b_out, gamma, beta, _trace=False):
    x = np.asarray(x, dtype=np.float32)
    wqkvT = np.ascontiguousarray(np.asarray(w_qkv, np.float16).T)   # [256, 1536]
    woutT = np.asarray(w_out, np.float16).T                         # [512, 256]
    # [128, 4*256]: woutT[k4*128+p, c] -> [p, k4*256+c]
    woutp = np.ascontiguousarray(
        woutT.reshape(4, 128, C).transpose(1, 0, 2).reshape(128, 4 * C))
    # gbuf cols: gamma m=0, gamma m=1, beta m=0, beta m=1
    g2 = np.asarray(gamma, np.float32).reshape(CT, 128).T
    b2 = np.asarray(beta, np.float32).reshape(CT, 128).T
    gbuf = np.ascontiguousarray(np.concatenate([g2, b2], axis=1))
    # b_out is intentionally unused: BatchNorm's mean subtraction cancels any
    # per-channel constant added before it, exactly.

    btot, c, hh, ww = x.shape
    assert (btot, c, hh * ww) == (B * N_CORES, C, NPIX)
    xf = x.reshape(btot, C, NPIX)

    nc = _build()
    in_maps = []
    for core in range(N_CORES):
        in_maps.append({
            "x": np.ascontiguousarray(
                xf[B * core:B * (core + 1)]).astype(np.float16),
            "wqkvT": wqkvT,
            "woutT": woutp,
            "gbuf": gbuf,
        })
    res = run_bass_kernel_spmd(nc, in_maps, core_ids=list(range(N_CORES)),
                               trace=_trace)
    y = np.concatenate([np.asarray(res.results[core]["y"], np.float32)
                        for core in range(N_CORES)], axis=0)
    out = y.reshape(btot, C, hh, ww)
    if _trace:
        kernel.last_result = res
    return out


# revision 42
# speedup vs baseline: 1.3071x; 1.0396x over previous
"""Trainium2 Bass kernel for nn_AttentionBlock (linear attention + BatchNorm).

Math (per batch, c=256 channels, n=1024 pixels, 8 heads x 64 dims):
  qkv = w_qkv @ x                      [1536, n]
  q   = softmax(q, axis=d) * d^-0.5    (per head, over the 64 head-dims)
  k   = softmax(k, axis=n)             (per head-dim, over pixels)
  ctx = k @ (v/n)^T                    [d, e] per head
  out = ctx^T @ q                      [e, n] per head
  y   = BatchNorm(w_out @ out + b_out) (batch stats over (b, n) per channel)

Sharding: data-parallel over batch across 8 cores (4 batches each); BN batch
stats are combined with a tiny AllReduce (2 floats per channel). b_out is
skipped: BatchNorm's mean subtraction cancels any per-channel constant exactly.

Device layouts (per batch):
  x      [c, n]           c on partitions (2 tiles, fp16)
  kv     [n, (k|v)]       per n-tile one 2-bank PSUM tile: k cols 0:512,
                          v cols 512:1024.  exp(k) -> expk fp16 (ACT);
                          v -> vx [128,8,65] fp16 with a ones column.
  ctx_h  [d, e+1]         contraction over n (8 chunks, PSUM accum, 4 banks);
                          col 64 = Zk (ones column) -> per-partition norm
  q      [(h d), n]       one 2-bank PSUM tile per q-tile; ONE exp -> expq
  Zq     [(h d), n]       block-mask matmul; recip folds SCALE and 1/n
  out_h  [e, n]           lhsT=cs, rhs=expq, quadrant-packed pairs of heads,
                          2-bank PSUM; normalized by recipb -> os fp16
  final  [c, n]           lhsT=w_out^T; fs fp16 resident; bn_stats per (b,m);
                          AllReduce of packed (mean, E[x^2]); normalize in
                          place; DMA out fp16.
"""

import os
import sys

import numpy as np

for _p in ("/opt/trn_rl_repo", "/root/.axon_site/_ro/trn_rl_repo"):
    if os.path.isdir(_p) and _p not in sys.path:
        sys.path.insert(0, _p)

import concourse.bacc as bacc
import concourse.tile as tile
from concourse import mybir
from concourse.bass_utils import run_bass_kernel_spmd

F32 = mybir.dt.float32
FP16 = mybir.dt.float16
AF = mybir.ActivationFunctionType
ALU = mybir.AluOpType

N_CORES = 8
# B is overridable for cheap simulator runs (BASS_ATTN_B=1 -> 8 batches total).
B = int(os.environ.get("BASS_ATTN_B", "4"))  # batches per core
C = 256          # channels
NPIX = 1024      # pixels (32*32)
H = 8            # heads
D = 64           # head dim
HID = H * D      # 512
NT = NPIX // 128  # 8 n-tiles
CT = C // 128     # 2 c-tiles
QT = HID // 128   # 4 q-tiles
SCALE = D ** -0.5
BN_EPS = 1e-5
# Zq-broadcast matmul uses this instead of 1.0 so reciprocal(Zqb) directly
# yields SCALE / (n * Zq), folding the softmax scale and the v/n factor.
MASKVAL = NPIX / SCALE
N_WARM_MM = 24   # junk matmuls at t=0 keep PE busy so HAM un-throttles early


DEBUG = os.environ.get("BASS_ATTN_DEBUG") == "1"


def _emit(tc, x, wqkv, wout, gbuf, y, dbg=None):
    nc = tc.nc
    from contextlib import ExitStack
    ctx_stack = ExitStack()
    with ctx_stack:
        const = ctx_stack.enter_context(tc.tile_pool(name="const", bufs=1))
        kvsb = ctx_stack.enter_context(tc.tile_pool(name="kvsb", bufs=3))
        vxp = ctx_stack.enter_context(tc.tile_pool(name="vxp", bufs=3))
        qpool = ctx_stack.enter_context(tc.tile_pool(name="qpool", bufs=3))
        rpool = ctx_stack.enter_context(tc.tile_pool(name="rpool", bufs=3))
        cpool = ctx_stack.enter_context(tc.tile_pool(name="cpool", bufs=5))
        opool = ctx_stack.enter_context(tc.tile_pool(name="opool", bufs=6))
        fpool = ctx_stack.enter_context(tc.tile_pool(name="fpool", bufs=2 * B))
        small = ctx_stack.enter_context(tc.tile_pool(name="small", bufs=8))
        stats_p = ctx_stack.enter_context(tc.tile_pool(name="statsp", bufs=1))
        # PSUM: pbig 2 x 2-bank tiles + pctx 4 x 1-bank tiles = 8 banks
        pbig = ctx_stack.enter_context(
            tc.tile_pool(name="pbig", bufs=2, space="PSUM"))
        pctx = ctx_stack.enter_context(
            tc.tile_pool(name="pctx", bufs=4, space="PSUM"))
        dpool = ctx_stack.enter_context(
            tc.tile_pool(name="dram", bufs=1, space="DRAM"))

        # ---- constants / warmup ----
        eps_sb = const.tile([128, 1], F32, name="eps")
        nc.vector.memset(eps_sb, BN_EPS)
        # load the Exp table set while input DMAs are in flight
        warm_ex = small.tile([1, 1], F32, name="warmex")
        nc.scalar.activation(out=warm_ex, in_=eps_sb[0:1, :], func=AF.Exp)

        bmask = const.tile([128, 128], FP16, name="bmask")
        nc.vector.memset(bmask, 0.0)
        nc.vector.memset(bmask[0:64, 0:64], MASKVAL)
        nc.vector.memset(bmask[64:128, 64:128], MASKVAL)

        # x batch 0 + kv weight columns first (they gate the first matmul)
        xin = [[None] * CT for _ in range(B)]
        wqkv_sb = [const.tile([128, 3 * HID], FP16, name=f"wqkv{kc}")
                   for kc in range(CT)]
        for kc in range(CT):
            xt = const.tile([128, NPIX], FP16, name=f"x0_{kc}")
            nc.sync.dma_start(out=xt, in_=x[0, 128 * kc:128 * (kc + 1), :])
            xin[0][kc] = xt
            nc.sync.dma_start(out=wqkv_sb[kc][:, HID:3 * HID],
                              in_=wqkv[128 * kc:128 * (kc + 1), HID:3 * HID])
        for kc in range(CT):
            nc.sync.dma_start(out=wqkv_sb[kc][:, 0:HID],
                              in_=wqkv[128 * kc:128 * (kc + 1), 0:HID])

        # junk matmuls: keep PE busy from ~t=0 so the HAM clock-gate opens
        # (K=8/8) before the first real matmul instead of ~10us in
        jw = pbig.tile([128, NPIX], F32, name="junk", tag="pb")
        for i in range(N_WARM_MM):
            nc.tensor.matmul(jw[:, 0:128], lhsT=bmask, rhs=bmask,
                             start=True, stop=True)

        # remaining weights / inputs on other queues
        wout_sb = const.tile([128, 4 * C], FP16, name="wout")
        nc.scalar.dma_start(out=wout_sb, in_=wout)
        for b in range(1, B):
            for kc in range(CT):
                xt = const.tile([128, NPIX], FP16, name=f"x{b}_{kc}")
                nc.scalar.dma_start(
                    out=xt, in_=x[b, 128 * kc:128 * (kc + 1), :])
                xin[b][kc] = xt
        gb_sb = const.tile([128, 4], F32, name="gb")
        nc.scalar.dma_start(out=gb_sb, in_=gbuf)

        # dummy collective issued up front: the first AllReduce pays a
        # ~24us+ ncfw rendezvous; running it early overlaps that with compute
        no_cc = os.environ.get("BASS_ATTN_NO_CC") == "1"
        if not no_cc:
            wrm_i = dpool.tile([128, 1], F32, name="wrm_i")
            wrm_o = dpool.tile([128, 1], F32, name="wrm_o")
            nc.sync.dma_start(out=wrm_i, in_=eps_sb)
            nc.gpsimd.collective_compute(
                "AllReduce", ALU.add,
                replica_groups=[list(range(N_CORES))],
                ins=[wrm_i.opt()], outs=[wrm_o.opt()])

        stats_sb = [stats_p.tile([128, 2 * B, 6], F32, name=f"stats{m}")
                    for m in range(CT)]
        final_sb = [[None] * CT for _ in range(B)]

        # block-diagonal stationary tiles for the out matmuls: off-diagonal
        # 64x64 blocks stay zero forever, so memset each rotating buffer once
        N_BD = 5
        bd_bufs = [cpool.tile([128, 128], FP16, name=f"bd{i}")
                   for i in range(N_BD)]
        for t_ in bd_bufs:
            nc.vector.memset(t_, 0.0)

        # deferred final-projection emitter: interleaved into the next
        # batch's kv phase so the PE never drains at batch boundaries
        def emit_final(b):
            for m in range(CT):
                fs = fpool.tile([128, NPIX], FP16, name="final")
                for nch in range(2):
                    fh = pctx.tile([128, 512], F32, name="fp", tag="pc")
                    for k4 in range(QT):
                        nc.tensor.matmul(
                            fh,
                            lhsT=wout_sb[:, k4 * C + 128 * m:
                                         k4 * C + 128 * (m + 1)],
                            rhs=os_sb[b][k4][:, 512 * nch:512 * (nch + 1)],
                            start=(k4 == 0), stop=(k4 == QT - 1))
                    fsh = fs[:, 512 * nch:512 * (nch + 1)]
                    nc.scalar.copy(fsh, fh)
                    nc.vector.bn_stats(
                        out=stats_sb[m][:, 2 * b + nch, :], in_=fsh)
                final_sb[b][m] = fs

        os_sb = [[None] * QT for _ in range(B)]

        for b in range(B):
            xc = xin[b]

            # ---- KV projection + context accumulation over n-chunks ----
            ctxu = [pctx.tile([128, 512], F32, name="ctxu", tag="pc")
                    for _ in range(4)]
            kvp = [None] * NT

            def emit_kv_mm(t):
                # kc outer: consecutive matmul pairs share the stationary
                # operand (the x chunk); k and v halves are separate
                # accumulation groups in separate banks
                hp = pbig.tile([128, NPIX], F32, name="kvp", tag="pb")
                for kc in range(CT):
                    for nch in range(2):
                        nc.tensor.matmul(
                            hp[:, 512 * nch:512 * (nch + 1)],
                            lhsT=xc[kc][:, 128 * t:128 * (t + 1)],
                            rhs=wqkv_sb[kc][:, HID + 512 * nch:
                                            HID + 512 * (nch + 1)],
                            start=(kc == 0), stop=(kc == CT - 1))
                kvp[t] = hp

            def emit_kv_drain(t):
                ek = kvsb.tile([128, HID], FP16, name="expk")
                nc.scalar.activation(out=ek, in_=kvp[t][:, 0:512],
                                     func=AF.Exp)
                vx = vxp.tile([128, H, D + 1], FP16, name="vx")
                nc.vector.memset(vx[:, :, D:D + 1], 1.0)
                # alternate the v drain between DVE and ACT: both engines
                # run near-saturated and this balances their load
                vsrc = kvp[t][:, 512:1024].rearrange("p (h e) -> p h e", h=H)
                if t % 2 == 0:
                    nc.vector.tensor_copy(vx[:, :, 0:D], vsrc)
                else:
                    nc.scalar.copy(vx[:, :, 0:D], vsrc)
                return ek, vx

            def emit_ctx_mm(t, ek, vx):
                # one matmul per head-pair: out [128, 130] holds
                # (d=h0 rows x [e_h0 | Zk_h0] cols 0:65) and
                # (d=h1 rows x [e_h1 | Zk_h1] cols 65:130); the cross-head
                # blocks are computed but never read.
                for pr in range(4):
                    nc.tensor.matmul(
                        ctxu[pr][:, 0:2 * (D + 1)],
                        lhsT=ek[:, 128 * pr:128 * (pr + 1)],
                        rhs=vx[:, 2 * pr:2 * (pr + 1), :],
                        start=(t == 0), stop=(t == NT - 1))

            # software-pipelined: kv matmuls for t+1 are emitted before the
            # ctx matmuls for t, so the PE queue never stalls on the drains.
            # The previous batch's final projection is interleaved here too.
            drains = [None] * NT
            emit_kv_mm(0)
            if b > 0:
                emit_final(b - 1)

            for t in range(NT):
                if t + 1 < NT:
                    emit_kv_mm(t + 1)
                drains[t] = emit_kv_drain(t)
                emit_ctx_mm(t, *drains[t])

            # ---- context normalization (per-partition Zk) into the
            # block-diagonal out-matmul stationary ----
            cs = []
            for pr in range(4):
                if DEBUG and b == 0:
                    ct = small.tile([128, 2 * (D + 1)], F32, name="dbgct")
                    nc.vector.tensor_copy(ct, ctxu[pr][:, 0:2 * (D + 1)])
                    nc.sync.dma_start(out=dbg["ctx"][pr], in_=ct)
                rz = small.tile([128, 1], F32, name="rz")
                # the cross-head block's ones column at col 64 carries Zk for
                # BOTH heads (rows 0:64 = h0, rows 64:128 = h1)
                nc.vector.reciprocal_approx_fast(
                    out=rz, in_=ctxu[pr][:, D:D + 1])
                bd = bd_bufs[(b * 4 + pr) % N_BD]
                nc.vector.tensor_scalar_mul(
                    bd[0:64, 0:64], in0=ctxu[pr][0:64, 0:D],
                    scalar1=rz[0:64, :])
                nc.vector.tensor_scalar_mul(
                    bd[64:128, 64:128], in0=ctxu[pr][64:128, D + 1:2 * D + 1],
                    scalar1=rz[64:128, :])
                if DEBUG and b == 0:
                    nc.sync.dma_start(out=dbg["rz"][pr], in_=rz)
                cs.append(bd)

            # ---- Q projection, exp, Zq block-broadcast, reciprocal ----
            expq, recipb = [None] * QT, [None] * QT

            def emit_q_mm(t):
                qp = pbig.tile([128, NPIX], F32, name="qp", tag="pb")
                for kc in range(CT):
                    for nch in range(2):
                        nc.tensor.matmul(
                            qp[:, 512 * nch:512 * (nch + 1)],
                            lhsT=wqkv_sb[kc][:, 128 * t:128 * (t + 1)],
                            rhs=xc[kc][:, 512 * nch:512 * (nch + 1)],
                            start=(kc == 0), stop=(kc == CT - 1))
                return qp

            def emit_q_drain(t, qp):
                eq = qpool.tile([128, NPIX], FP16, name="expq")
                nc.scalar.activation(out=eq, in_=qp, func=AF.Exp)
                expq[t] = eq

            def emit_zq(t):
                rb = rpool.tile([128, NPIX], F32, name="recipb")
                for nch in range(2):
                    zp = pctx.tile([128, 512], F32, name="zq", tag="pc")
                    nc.tensor.matmul(
                        zp, lhsT=bmask,
                        rhs=expq[t][:, 512 * nch:512 * (nch + 1)],
                        start=True, stop=True)
                    nc.vector.reciprocal_approx_fast(
                        out=rb[:, 512 * nch:512 * (nch + 1)], in_=zp)
                recipb[t] = rb

            def emit_out(t):
                oh = pbig.tile([128, NPIX], F32, name="ou", tag="pb")
                for nch in range(2):
                    # block-diagonal stationary: one full-width matmul
                    # covers both heads of the pair
                    nc.tensor.matmul(
                        oh[:, 512 * nch:512 * (nch + 1)],
                        lhsT=cs[t],
                        rhs=expq[t][:, 512 * nch:512 * (nch + 1)],
                        start=True, stop=True)
                os_ = opool.tile([128, NPIX], FP16, name="outsb")
                nc.vector.tensor_mul(os_, oh, recipb[t])
                os_sb[b][t] = os_

            # pipeline: qmm(t+1) is emitted before zq(t) (which waits on the
            # ACT exp of tile t), and out(t) trails by one more stage
            qps = [None] * QT
            qps[0] = emit_q_mm(0)
            for t in range(QT):
                emit_q_drain(t, qps[t])
                if t + 1 < QT:
                    qps[t + 1] = emit_q_mm(t + 1)
                emit_zq(t)
                if t >= 1:
                    emit_out(t - 1)
            emit_out(QT - 1)

            if DEBUG and b == 0:
                nc.sync.dma_start(out=dbg["expk"], in_=drains[0][0])
                nc.sync.dma_start(out=dbg["vx"], in_=drains[0][1])
                for pr in range(4):
                    nc.sync.dma_start(out=dbg["bd"][pr], in_=cs[pr])
                    nc.sync.dma_start(out=dbg["os"][pr], in_=os_sb[b][pr])
                    nc.sync.dma_start(out=dbg["rb"][pr], in_=recipb[pr])
                    nc.sync.dma_start(out=dbg["eq"][pr], in_=expq[pr])


        emit_final(B - 1)

        # ---- batch-norm: aggregate, exchange stats via raw remote DMA ----
        # switch the ACT table to the sqrt set while PE still runs the last
        # final-proj matmuls, so the tail's Sqrt doesn't pay the ~2.7us load
        warm_sq = small.tile([1, 1], F32, name="warmsq")
        nc.scalar.activation(out=warm_sq, in_=eps_sb[0:1, :], func=AF.Sqrt)
        pk4 = small.tile([128, 2 * CT], F32, name="pk4")
        for m in range(CT):
            mv = small.tile([128, 2], F32, name="mv")
            nc.vector.bn_aggr(out=mv, in_=stats_sb[m])
            pk = pk4[:, 2 * m:2 * (m + 1)]
            nc.vector.tensor_mul(pk[:, 1:2], mv[:, 0:1], mv[:, 0:1])
            nc.vector.tensor_add(pk[:, 1:2], pk[:, 1:2], mv[:, 1:2])
            nc.vector.tensor_copy(pk[:, 0:1], mv[:, 0:1])
            nc.vector.tensor_scalar_mul(pk, in0=pk, scalar1=1.0 / N_CORES)
        if no_cc:
            gst = small.tile([128, 2 * CT], F32, name="gst")
            nc.vector.tensor_copy(gst, pk4)
            # in-Tile tail (simulator path only)
            g2v = gst.rearrange("p (m k) -> p m k", k=2)
            gmean2 = g2v[:, :, 0]
            gex22 = g2v[:, :, 1]
            var2 = small.tile([128, 2], F32, name="var2")
            nc.vector.tensor_mul(var2, gmean2, gmean2)
            nc.vector.tensor_sub(var2, gex22, var2)
            std2 = small.tile([128, 2], F32, name="std2")
            nc.scalar.activation(out=std2, in_=var2, func=AF.Sqrt, bias=eps_sb)
            rsg2 = small.tile([128, 2], F32, name="rsg2")
            nc.vector.reciprocal_approx_fast(out=rsg2, in_=std2)
            nc.vector.tensor_mul(rsg2, rsg2, gb_sb[:, 0:2])
            sh2 = small.tile([128, 2], F32, name="sh2")
            nc.vector.tensor_mul(sh2, gmean2, rsg2)
            nc.vector.tensor_sub(sh2, gb_sb[:, 2:4], sh2)
            for m in range(CT):
                for b in range(B):
                    fs = final_sb[b][m]
                    nc.vector.tensor_scalar(
                        out=fs, in0=fs, scalar1=rsg2[:, m:m + 1],
                        scalar2=sh2[:, m:m + 1], op0=ALU.mult, op1=ALU.add)
                    nc.sync.dma_start(
                        out=y[b, 128 * m:128 * (m + 1), :], in_=fs)
            return None

        ccin = dpool.tile([128, 2 * CT], F32, name="ccin")
        ccout = dpool.tile([128, 2 * CT], F32, name="ccout")
        nc.sync.dma_start(out=ccin, in_=pk4)
        nc.gpsimd.collective_compute(
            "AllReduce", ALU.add,
            replica_groups=[list(range(N_CORES))],
            ins=[ccin.opt()], outs=[ccout.opt()])
        gst = small.tile([128, 2 * CT], F32, name="gst")
        nc.sync.dma_start(out=gst, in_=ccout)
        g2v = gst.rearrange("p (m k) -> p m k", k=2)
        gmean2 = g2v[:, :, 0]
        gex22 = g2v[:, :, 1]
        var2 = small.tile([128, 2], F32, name="var2")
        nc.vector.tensor_mul(var2, gmean2, gmean2)
        nc.vector.tensor_sub(var2, gex22, var2)
        std2 = small.tile([128, 2], F32, name="std2")
        nc.scalar.activation(out=std2, in_=var2, func=AF.Sqrt, bias=eps_sb)
        rsg2 = small.tile([128, 2], F32, name="rsg2")
        nc.vector.reciprocal_approx_fast(out=rsg2, in_=std2)
        nc.vector.tensor_mul(rsg2, rsg2, gb_sb[:, 0:2])
        sh2 = small.tile([128, 2], F32, name="sh2")
        nc.vector.tensor_mul(sh2, gmean2, rsg2)
        nc.vector.tensor_sub(sh2, gb_sb[:, 2:4], sh2)
        dmaq = [nc.sync, nc.gpsimd, nc.scalar, nc.gpsimd]
        for m in range(CT):
            rsg = rsg2[:, m:m + 1]
            sh = sh2[:, m:m + 1]
            for b in range(B):
                fs = final_sb[b][m]
                if b == 1:
                    nc.scalar.activation(
                        out=fs, in_=fs, func=AF.Identity,
                        bias=sh, scale=rsg)
                else:
                    nc.vector.tensor_scalar(
                        out=fs, in0=fs, scalar1=rsg, scalar2=sh,
                        op0=ALU.mult, op1=ALU.add)
                dmaq[(2 * m + b) % 4].dma_start(
                    out=y[b, 128 * m:128 * (m + 1), :], in_=fs)
        return None


_CACHE = {}


def _build():
    if "nc" in _CACHE:
        return _CACHE["nc"]
    nc = bacc.Bacc("TRN2", target_bir_lowering=False, debug=False,
                   enable_asserts=True, num_devices=N_CORES)
    x = nc.dram_tensor("x", [B, C, NPIX], FP16, kind="ExternalInput").ap()
    wqkv = nc.dram_tensor("wqkvT", [C, 3 * HID], FP16,
                          kind="ExternalInput").ap()
    wout = nc.dram_tensor("woutT", [128, 4 * C], FP16,
                          kind="ExternalInput").ap()
    gbuf = nc.dram_tensor("gbuf", [128, 4], F32, kind="ExternalInput").ap()
    y = nc.dram_tensor("y", [B, C, NPIX], FP16, kind="ExternalOutput").ap()
    dbg = None
    if DEBUG:
        dbg = {
            "expk": nc.dram_tensor("d_expk", [128, HID], FP16,
                                   kind="ExternalOutput").ap(),
            "vx": nc.dram_tensor("d_vx", [128, H, D + 1], FP16,
                                 kind="ExternalOutput").ap(),
            "bd": [nc.dram_tensor(f"d_bd{p}", [128, 128], FP16,
                                  kind="ExternalOutput").ap() for p in range(4)],
            "os": [nc.dram_tensor(f"d_os{p}", [128, NPIX], FP16,
                                  kind="ExternalOutput").ap() for p in range(4)],
            "rb": [nc.dram_tensor(f"d_rb{p}", [128, NPIX], F32,
                                  kind="ExternalOutput").ap() for p in range(4)],
            "eq": [nc.dram_tensor(f"d_eq{p}", [128, NPIX], FP16,
                                  kind="ExternalOutput").ap() for p in range(4)],
            "ctx": [nc.dram_tensor(f"d_ctx{p}", [128, 2 * (D + 1)], F32,
                                   kind="ExternalOutput").ap() for p in range(4)],
            "rz": [nc.dram_tensor(f"d_rz{p}", [128, 1], F32,
                                  kind="ExternalOutput").ap() for p in range(4)],
        }
    with tile.TileContext(nc) as tc:
        _emit(tc, x, wqkv, wout, gbuf, y, dbg)
    nc.compile()
    _CACHE["nc"] = nc
    return nc


def kernel(x, w_qkv, w_out, b_out, gamma, beta, _trace=False):
    x = np.asarray(x, dtype=np.float32)
    wqkvT = np.ascontiguousarray(np.asarray(w_qkv, np.float16).T)   # [256, 1536]
    woutT = np.asarray(w_out, np.float16).T                         # [512, 256]
    # [128, 4*256]: woutT[k4*128+p, c] -> [p, k4*256+c]
    woutp = np.ascontiguousarray(
        woutT.reshape(4, 128, C).transpose(1, 0, 2).reshape(128, 4 * C))
    # gbuf cols: gamma m=0, gamma m=1, beta m=0, beta m=1
    g2 = np.asarray(gamma, np.float32).reshape(CT, 128).T
    b2 = np.asarray(beta, np.float32).reshape(CT, 128).T
    gbuf = np.ascontiguousarray(np.concatenate([g2, b2], axis=1))
    # b_out is intentionally unused: BatchNorm's mean subtraction cancels any
    # per-channel constant added before it, exactly.

    btot, c, hh, ww = x.shape
    assert (btot, c, hh * ww) == (B * N_CORES, C, NPIX)
    xf = x.reshape(btot, C, NPIX)

    nc = _build()
    in_maps = []
    for core in range(N_CORES):
        in_maps.append({
            "x": np.ascontiguousarray(
                xf[B * core:B * (core + 1)]).astype(np.float16),
            "wqkvT": wqkvT,
            "woutT": woutp,
            "gbuf": gbuf,
        })
    res = run_bass_kernel_spmd(nc, in_maps, core_ids=list(range(N_CORES)),
                               trace=_trace)
    y = np.concatenate([np.asarray(res.results[core]["y"], np.float32)
                        for core in range(N_CORES)], axis=0)
    out = y.reshape(btot, C, hh, ww)
    if _trace:
        kernel.last_result = res
    return out
